# revision 1
# baseline (speedup 1.0000x reference)
"""Trainium2 Bass kernel for the 4-layer GCN diffusion denoiser (gnn_message_passing).

Strategy (8 NeuronCores, SPMD single program):
  - Nodes sharded 12500/core (padded to 12544 = 98*128). Edges routed to the core
    owning their dst node, bucketed into 512-node windows.
  - Per layer, per-node features Hs = dinv * (X @ W) are stored as a bf16
    [100352, 128] table (row-padded feature dim), AllGather'ed across cores.
  - Aggregation per 512-node window: bulk indirect row gathers (dma_gather,
    int16 indices into 4 x 25088-row bucket views), segment-sum via PE matmuls
    against on-device-built one-hot matrices (iota == dst_local) * dinv_dst,
    self-loop via HsSelf x diag(dinv) matmul, Silu(agg + bias) on ScalarE.
  - The next layer's H-matmul consumes the transposed activation tile directly
    (lhsT = x'^T), so no transposes are needed except for the final output.

All cross-core communication is 4 AllGathers (one per layer boundary).
"""

import math
import sys
import types

import numpy as np

_N, _E, _D, _G = 100000, 1000000, 64, 128
_NCORES = 8
_SL = _N // _NCORES          # 12500 real nodes per core
_SLP = 12544                 # padded per-core slice (98*128)
_NP = _SLP * _NCORES         # 100352 padded table rows
_NBUCK = 4
_BUCK = _NP // _NBUCK        # 25088 rows per gather bucket (int16-addressable)
_WIN = 512
_NWIN = (_SLP + _WIN - 1) // _WIN    # 25 windows (last is 256 nodes)
_NSUB = _SLP // 128          # 98 sub-tiles of 128 nodes
_F = 128
_PAD_DST = 1000.0
_MAX_CHUNKS_PER_CALL = 4     # <=512 rows per dma_gather (descriptor ring limit)

_compiled = {}


def _install_profile_shim():
    """Register the NTFF profile hook missing from this image's antenv."""
    try:
        import antenv
        from trn_agent_boot.trn_boot import _ntff_profile_via_ctypes
    except ImportError:
        return
    if "antenv.axon_hooks" in sys.modules:
        return
    mod = types.ModuleType("antenv.axon_hooks")
    hook = _ntff_profile_via_ctypes("/opt/axon/libaxon_pjrt.so")
    mod.get_axon_ntff_profile_hook = lambda: hook
    mod.set_axon_ntff_profile_hook = lambda h: None
    sys.modules["antenv.axon_hooks"] = mod
    antenv.axon_hooks = mod


def _prep(inputs):
    """Host-side metadata build: edge bucketing, gather indices, selectors."""
    src = np.asarray(inputs["edge_index"][0], dtype=np.int64)
    dst = np.asarray(inputs["edge_index"][1], dtype=np.int64)
    deg = np.bincount(dst, minlength=_N).astype(np.float32) + 1.0
    dinv = (1.0 / np.sqrt(deg)).astype(np.float32)

    node_ids = np.arange(_N)
    node_row = (_SLP * (node_ids // _SL) + (node_ids % _SL)).astype(np.int64)

    core_of = dst // _SL
    dloc = dst % _SL
    st_of = dloc // 128                      # sub-tile (128-node segment) in core
    dstloc = (dloc % 128).astype(np.float32)  # position within sub-tile
    srow = node_row[src]
    buck = srow // _BUCK
    idx16 = (srow % _BUCK).astype(np.int16)
    dinv_dst = dinv[dst]

    # slot order: core -> window -> bucket -> sub-tile -> edges
    w_of = st_of // 4
    stw = st_of % 4
    order = np.lexsort((stw, buck, w_of, core_of))
    buck_s, core_s, w_s, stw_s = buck[order], core_of[order], w_of[order], stw[order]
    idx16_s, dstloc_s, dinvd_s = idx16[order], dstloc[order], dinv_dst[order]

    key = (((core_s * _NWIN) + w_s) * _NBUCK + buck_s) * 4 + stw_s
    nkeys = _NCORES * _NWIN * _NBUCK * 4
    counts = np.bincount(key, minlength=nkeys)
    runlen = counts.reshape(_NCORES, _NWIN, _NBUCK, 4)
    # chunks per (window, bucket, sub-tile): uniform across cores
    cpb = np.maximum(1, np.ceil(runlen.max(axis=0) / 128).astype(np.int64))  # [NWIN, NBUCK, 4]
    # drop sub-tiles that don't exist (last window has only 2)
    for w in range(_NWIN):
        ws_ = min(_WIN, _SLP - w * _WIN)
        cpb[w, :, ws_ // 128:] = 0
    nchunks_w = cpb.sum(axis=(1, 2))
    tot_chunks = int(nchunks_w.sum())
    tot_slots = tot_chunks * 128

    call_off = np.zeros((_NWIN, _NBUCK, 4), np.int64)
    acc = 0
    chunk_meta = []   # per window: list of (bucket, sub-tile) per chunk, in slot order
    for w in range(_NWIN):
        cm = []
        for b in range(_NBUCK):
            for st in range(4):
                call_off[w, b, st] = acc
                acc += int(cpb[w, b, st]) * 128
                cm += [(b, st)] * int(cpb[w, b, st])
        chunk_meta.append(cm)
    assert acc == tot_slots

    run_start = np.zeros(nkeys + 1, np.int64)
    np.cumsum(counts, out=run_start[1:])

    is_norm = np.zeros(_N, bool)
    is_norm[np.asarray(inputs["train_norm"])] = True
    is_anm = np.zeros(_N, bool)
    is_anm[np.asarray(inputs["train_anm"])] = True

    noise = np.asarray(inputs["noise_x"], np.float32)
    t_val = float(np.asarray(inputs["t"]).reshape(-1)[0])
    half = _D // 2
    freqs = np.exp(
        np.arange(half, dtype=np.float32) * (-math.log(10000.0) / (half - 1))
    ).astype(np.float32)

    w_ = {m: np.asarray(inputs[m], np.float32) for m in
          ["w0", "b0", "w1", "b1", "w2", "b2", "w3", "b3",
           "time_w1", "time_b1", "time_w2", "time_b2", "label_emb"]}
    w1p = np.zeros((128, 128), np.float32); w1p[:, :64] = w_["w1"]
    w2p = np.zeros((128, 128), np.float32); w2p[:64, :] = w_["w2"]
    w3ap = np.zeros((128, 128), np.float32); w3ap[:, :64] = w_["w3"][:128]
    w3bp = np.zeros((128, 128), np.float32); w3bp[:, :64] = w_["w3"][128:]
    b1p = np.zeros((128, 1), np.float32); b1p[:64, 0] = w_["b1"]
    b3p = np.zeros((128, 1), np.float32); b3p[:64, 0] = w_["b3"]

    shared = {
        "w0": w_["w0"],                       # [64, 128]
        "w1p": w1p, "w2p": w2p, "w3ap": w3ap, "w3bp": w3bp,
        "b0c": w_["b0"].reshape(128, 1).astype(np.float32),
        "b1c": b1p,
        "b2c": w_["b2"].reshape(128, 1).astype(np.float32),
        "b3c": b3p,
        "tw1": w_["time_w1"], "tw2": w_["time_w2"],
        "tb1": w_["time_b1"].reshape(64, 1).astype(np.float32),
        "tb2": w_["time_b2"].reshape(64, 1).astype(np.float32),
        "e0row": w_["label_emb"][0].reshape(1, 64).astype(np.float32),
        "e1row": w_["label_emb"][1].reshape(1, 64).astype(np.float32),
        "rsin": (np.mod(t_val * freqs + np.pi, 2 * np.pi) - np.pi).reshape(32, 1).astype(np.float32),
        "rcos": (np.mod(t_val * freqs + np.pi / 2 + np.pi, 2 * np.pi) - np.pi).reshape(32, 1).astype(np.float32),
        "ones1": np.ones((1, 128), np.float32),
    }

    in_maps = []
    for k in range(_NCORES):
        idx_slots = np.zeros(tot_slots, np.int16)
        dstloc_slots = np.full(tot_slots, _PAD_DST, np.float32)
        dinvd_slots = np.zeros(tot_slots, np.float32)
        for w in range(_NWIN):
            for b in range(_NBUCK):
                for st in range(4):
                    if cpb[w, b, st] == 0:
                        continue
                    kk = (((k * _NWIN) + w) * _NBUCK + b) * 4 + st
                    s0, s1 = run_start[kk], run_start[kk + 1]
                    o = call_off[w, b, st]
                    idx_slots[o:o + (s1 - s0)] = idx16_s[s0:s1]
                    dstloc_slots[o:o + (s1 - s0)] = dstloc_s[s0:s1]
                    dinvd_slots[o:o + (s1 - s0)] = dinvd_s[s0:s1]
        wrapped = np.tile(idx_slots.reshape(-1, 16).T, (8, 1))
        dl = dstloc_slots.reshape(-1, 128).T.astype(np.float16)
        dvd = dinvd_slots.reshape(-1, 128).T.astype(np.float16)

        nodes = np.arange(_SLP) + k * _SL
        nodes_c = np.minimum(nodes, _N - 1)
        sd = dinv[nodes_c].copy()
        sd[np.arange(_SLP) >= _SL] = 1.0
        selfdinv = sd.reshape(_NSUB, 128).T.copy()

        s_n = np.zeros(_SLP, np.float32)
        s_a = np.zeros(_SLP, np.float32)
        real = np.arange(_SLP) < _SL
        s_n[real] = is_norm[nodes[real]]
        s_a[real] = is_anm[nodes[real]] & ~is_norm[nodes[real]]

        nz = np.zeros((_SLP, _D), np.float32)
        nz[:_SL] = noise[k * _SL:(k + 1) * _SL]

        m = dict(shared)
        m.update({
            "midx": wrapped,
            "mdstl": dl,
            "mdinvd": dvd,
            "mself": selfdinv,
            "ms0": s_n.reshape(_NSUB, 128).T.copy(),
            "ms1": s_a.reshape(_NSUB, 128).T.copy(),
            "noise": nz,
        })
        in_maps.append(m)

    return in_maps, cpb, call_off, nchunks_w, chunk_meta, tot_chunks, tot_slots


def _build(cpb, call_off, nchunks_w, chunk_meta, tot_chunks, tot_slots):
    import concourse.bass as bass
    import concourse.bacc as bacc
    import concourse.tile as tile
    from concourse import mybir
    from concourse.masks import make_identity

    f32 = mybir.dt.float32
    bf16 = mybir.dt.float16
    AT = mybir.ActivationFunctionType
    OP = mybir.AluOpType

    nc = bacc.Bacc("TRN2", target_bir_lowering=False, debug=False,
                   num_devices=_NCORES, dynamic_dma_scratch_size=32768,
                   num_swdge_queues=4)

    din = {}
    def dt_in(name, shape, dt):
        din[name] = nc.dram_tensor(name, list(shape), dt, kind="ExternalInput")
        return din[name]

    dt_in("noise", (_SLP, _D), f32)
    dt_in("midx", (128, tot_slots // 16), mybir.dt.int16)
    dt_in("mdstl", (128, tot_chunks), bf16)
    dt_in("mdinvd", (128, tot_chunks), bf16)
    dt_in("mself", (128, _NSUB), f32)
    dt_in("ms0", (128, _NSUB), f32)
    dt_in("ms1", (128, _NSUB), f32)
    dt_in("w0", (64, 128), f32)
    for nm in ["w1p", "w2p", "w3ap", "w3bp"]:
        dt_in(nm, (128, 128), f32)
    for nm in ["b0c", "b1c", "b2c", "b3c"]:
        dt_in(nm, (128, 1), f32)
    dt_in("tw1", (64, 64), f32)
    dt_in("tw2", (64, 64), f32)
    dt_in("tb1", (64, 1), f32)
    dt_in("tb2", (64, 1), f32)
    dt_in("e0row", (1, 64), f32)
    dt_in("e1row", (1, 64), f32)
    dt_in("rsin", (32, 1), f32)
    dt_in("rcos", (32, 1), f32)
    dt_in("ones1", (1, 128), f32)
    out_d = nc.dram_tensor("out", [_SLP, 64], f32, kind="ExternalOutput")

    wnames = ["w1p", "w2p", None]  # H-matmul weights for layers 0,1 (2 handled via w3a/w3b)
    bnames = ["b0c", "b1c", "b2c", "b3c"]

    def _interleave(dram_tile, row0, nst, F):
        ap = dram_tile[:]
        return bass.AP(ap.tensor, ap.offset + row0 * F,
                       [[F, 128], [128 * F, nst], [1, F]])

    qctr = [0]
    def next_q():
        q = qctr[0] % 4
        qctr[0] += 1
        return q

    with tile.TileContext(nc) as tc:
        with tc.tile_pool(name="consts", bufs=1) as cp, \
             tc.tile_pool(name="meta", bufs=1) as mp, \
             tc.tile_pool(name="dram", bufs=1, space="DRAM") as dram, \
             tc.tile_pool(name="g", bufs=3) as gp, \
             tc.tile_pool(name="oh", bufs=2) as ohp, \
             tc.tile_pool(name="small", bufs=4) as sp, \
             tc.tile_pool(name="xt", bufs=2) as xtp, \
             tc.tile_pool(name="psA", bufs=2, space="PSUM") as psA, \
             tc.tile_pool(name="psB", bufs=3, space="PSUM") as psB, \
             tc.tile_pool(name="psC", bufs=2, space="PSUM") as psC:

            # ---- constants / metadata into SBUF ----
            def load(name, shape, dt, pool=cp):
                t = pool.tile(list(shape), dt, tag=name, name=name)
                nc.sync.dma_start(out=t[:], in_=din[name].ap())
                return t

            idx_t = load("midx", (128, tot_slots // 16), mybir.dt.int16, mp)
            dstl_t = load("mdstl", (128, tot_chunks), bf16, mp)
            dinvd_t = load("mdinvd", (128, tot_chunks), bf16, mp)
            self_t = load("mself", (128, _NSUB), f32)
            s0_t = load("ms0", (128, _NSUB), f32)
            s1_t = load("ms1", (128, _NSUB), f32)
            w0_t = load("w0", (64, 128), f32)
            wl_t = {nm: load(nm, (128, 128), f32) for nm in ["w1p", "w2p", "w3ap", "w3bp"]}
            b_t = {nm: load(nm, (128, 1), f32) for nm in bnames}
            tw1_t = load("tw1", (64, 64), f32)
            tw2_t = load("tw2", (64, 64), f32)
            tb1_t = load("tb1", (64, 1), f32)
            tb2_t = load("tb2", (64, 1), f32)
            e0_t = load("e0row", (1, 64), f32)
            e1_t = load("e1row", (1, 64), f32)
            rsin_t = load("rsin", (32, 1), f32)
            rcos_t = load("rcos", (32, 1), f32)
            ones1_t = load("ones1", (1, 128), f32)

            iota_i = cp.tile([128, 128], mybir.dt.int32, tag="iotai", name="iotai")
            nc.gpsimd.iota(iota_i[:], pattern=[[1, 128]], base=0, channel_multiplier=0)
            iota_b = cp.tile([128, 128], bf16, tag="iotab", name="iotab")
            nc.vector.tensor_copy(iota_b[:], iota_i[:])
            self_h = cp.tile([128, _NSUB], bf16, tag="selfh", name="selfh")
            nc.vector.tensor_copy(self_h[:], self_t[:])
            eye_t = cp.tile([128, 128], f32, tag="eye", name="eye")
            make_identity(nc, eye_t[:])
            eye_h = cp.tile([128, 128], bf16, tag="eyeh", name="eyeh")
            nc.vector.tensor_copy(eye_h[:], eye_t[:])

            # ---- DRAM working buffers ----
            slice_d = [dram.tile([_SLP, _F], bf16, tag=f"slice{l}", name=f"slice{l}") for l in range(4)]
            full_d = [dram.tile([_NP, _F], bf16, tag=f"full{l}", name=f"full{l}") for l in range(4)]
            h0T_d = dram.tile([128, _SLP], f32, tag="h0T", name="h0T")

            # ---- timestep embedding ----
            sc_t = sp.tile([64, 1], f32, tag="tsc", name="tsc")
            nc.scalar.activation(sc_t[:32, :], rsin_t[:], AT.Sin)
            nc.scalar.activation(sc_t[32:64, :], rcos_t[:], AT.Sin)
            h1ps = psC.tile([64, 1], f32, tag="temb", name="h1ps")
            nc.tensor.matmul(h1ps[:], lhsT=tw1_t[:], rhs=sc_t[:], start=True, stop=True)
            h1_t = sp.tile([64, 1], f32, tag="th1", name="th1")
            nc.scalar.activation(h1_t[:], h1ps[:], AT.Silu, bias=tb1_t[:, :1])
            t2ps = psC.tile([64, 1], f32, tag="temb", name="t2ps")
            nc.tensor.matmul(t2ps[:], lhsT=tw2_t[:], rhs=h1_t[:], start=True, stop=True)
            tembT = sp.tile([64, 1], f32, tag="tembT", name="tembT")
            nc.vector.tensor_scalar(out=tembT[:], in0=t2ps[:], scalar1=tb2_t[:, :1],
                                    scalar2=None, op0=OP.add)
            trow_ps = psC.tile([1, 64], f32, tag="temb", name="trow_ps")
            nc.tensor.transpose(trow_ps[:], in_=tembT[:], identity=eye_t[:64, :64])
            trow_t = sp.tile([1, 64], f32, tag="trowS", name="trowS")
            nc.vector.tensor_copy(trow_t[:], trow_ps[:])
            rows_ps = psC.tile([128, 192], f32, tag="temb", name="rows_ps")
            nc.tensor.matmul(rows_ps[:, 0:64], lhsT=ones1_t[:], rhs=trow_t[:],
                             start=True, stop=True, skip_group_check=True)
            nc.tensor.matmul(rows_ps[:, 64:128], lhsT=ones1_t[:], rhs=e0_t[:],
                             start=True, stop=True, skip_group_check=True)
            nc.tensor.matmul(rows_ps[:, 128:192], lhsT=ones1_t[:], rhs=e1_t[:],
                             start=True, stop=True, skip_group_check=True)
            addrows = cp.tile([128, 192], f32, tag="addrows", name="addrows")
            nc.vector.tensor_copy(addrows[:], rows_ps[:])

            # ---- x0 phase: Hs0 = dinv * ((noise + temb + lab) @ w0) ----
            for st in range(_NSUB):
                nz = sp.tile([128, 64], f32, tag="nz", name="nz")
                nc.sync.dma_start(out=nz[:], in_=din["noise"].ap()[st * 128:(st + 1) * 128, :])
                x0 = sp.tile([128, 64], f32, tag="x0", name="x0")
                nc.vector.tensor_add(x0[:], nz[:], addrows[:, 0:64])
                lab = sp.tile([128, 64], f32, tag="lab", name="lab")
                nc.vector.tensor_scalar(out=lab[:], in0=addrows[:, 64:128],
                                        scalar1=s0_t[:, st:st + 1], scalar2=None, op0=OP.mult)
                nc.vector.tensor_add(x0[:], x0[:], lab[:])
                nc.vector.tensor_scalar(out=lab[:], in0=addrows[:, 128:192],
                                        scalar1=s1_t[:, st:st + 1], scalar2=None, op0=OP.mult)
                nc.vector.tensor_add(x0[:], x0[:], lab[:])
                x0T_ps = psB.tile([64, 128], f32, tag="mm128", name="x0T_ps")
                nc.tensor.transpose(x0T_ps[:], in_=x0[:], identity=eye_t[:])
                x0T = sp.tile([64, 128], f32, tag="x0Ts", name="x0Ts")
                nc.vector.tensor_copy(x0T[:], x0T_ps[:])
                hps = psB.tile([128, 128], f32, tag="mm128", name="hps")
                nc.tensor.matmul(hps[:], lhsT=x0T[:], rhs=w0_t[:], start=True, stop=True)
                hs0 = sp.tile([128, _F], bf16, tag="hsout", name="hsout")
                nc.vector.tensor_scalar(out=hs0[:], in0=hps[:],
                                        scalar1=self_t[:, st:st + 1], scalar2=None, op0=OP.mult)
                nc.sync.dma_start(out=slice_d[0][st * 128:(st + 1) * 128, :], in_=hs0[:])

            # ---- layers ----
            for layer in range(4):
                nc.gpsimd.collective_compute(
                    "AllGather", mybir.AluOpType.bypass,
                    replica_groups=[list(range(_NCORES))],
                    ins=[slice_d[layer].opt()], outs=[full_d[layer].opt()],
                )
                full = full_d[layer]
                for w in range(_NWIN):
                    ws = min(_WIN, _SLP - w * _WIN)
                    ncw = int(nchunks_w[w])
                    cbase = int(np.sum(nchunks_w[:w]))
                    g = gp.tile([128, ncw, _F], bf16, tag="g", name="g")
                    # gather calls: per bucket, <=4-chunk calls over the
                    # concatenated (padded) sub-tile runs
                    crel = 0
                    for b in range(_NBUCK):
                        nch = int(cpb[w, b, :].sum())
                        o16 = int(call_off[w, b, 0]) // 16
                        done = 0
                        while done < nch:
                            cc = min(_MAX_CHUNKS_PER_CALL, nch - done)
                            ni = cc * 128
                            nc.gpsimd.dma_gather(
                                out_ap=g[:, crel + done:crel + done + cc, :],
                                in_ap=full[b * _BUCK:(b + 1) * _BUCK, :],
                                idxs_ap=idx_t[:, o16 + done * 8: o16 + done * 8 + ni // 16],
                                num_idxs=ni, num_idxs_reg=ni, elem_size=_F,
                                queue_num=next_q(),
                            )
                            done += cc
                        crel += nch
                    nst = ws // 128
                    agg = psA.tile([128, ws], f32, tag="agg", name="agg")
                    # batched one-hot: (iota == dstloc) * dinv for all chunks at once
                    iota_rep = bass.AP(iota_b[:].tensor, iota_b[:].offset,
                                       [list(iota_b[:].ap[0]), [0, ncw], [1, 128]])
                    ohA = ohp.tile([128, ncw, 128], bf16, tag="ohA", name="ohA")
                    ohB = ohp.tile([128, ncw, 128], bf16, tag="ohB", name="ohB")
                    nc.vector.tensor_tensor(
                        out=ohA[:], in0=iota_rep,
                        in1=dstl_t[:, cbase:cbase + ncw][:, :, None].to_broadcast([128, ncw, 128]),
                        op=OP.is_equal)
                    nc.vector.tensor_tensor(
                        out=ohB[:], in0=ohA[:],
                        in1=dinvd_t[:, cbase:cbase + ncw][:, :, None].to_broadcast([128, ncw, 128]),
                        op=OP.mult)
                    for c in range(ncw):
                        b_c, st_c = chunk_meta[w][c]
                        # start=True clears the whole bank's has_written bits;
                        # start=False on cleared elements overwrites and sets
                        # the bit, so only the window's first matmul starts.
                        nc.tensor.matmul(agg[:, st_c * 128:(st_c + 1) * 128],
                                         lhsT=g[:, c, :], rhs=ohB[:, c, :],
                                         start=(c == 0), stop=False,
                                         skip_group_check=True)
                    # self-loop terms: batched load of my slice rows + eye*dinv rhs
                    hsb = sp.tile([128, nst, _F], bf16, tag="hself", name="hself")
                    nc.sync.dma_start(
                        out=hsb[:],
                        in_=_interleave(slice_d[layer], w * _WIN, nst, _F))
                    srhs = sp.tile([128, nst, 128], bf16, tag="srhs", name="srhs")
                    eye_rep = bass.AP(eye_h[:].tensor, eye_h[:].offset,
                                      [list(eye_h[:].ap[0]), [0, nst], [1, 128]])
                    nc.vector.tensor_tensor(
                        out=srhs[:], in0=eye_rep,
                        in1=self_h[:, 4 * w:4 * w + nst][:, :, None].to_broadcast([128, nst, 128]),
                        op=OP.mult)
                    for st in range(nst):
                        nc.tensor.matmul(agg[:, st * 128:(st + 1) * 128],
                                         lhsT=hsb[:, st, :], rhs=srhs[:, st, :],
                                         start=False, stop=(st == nst - 1),
                                         skip_group_check=True)
                    xT = xtp.tile([128, ws], f32, tag="xT", name="xT")
                    nc.scalar.activation(xT[:], agg[:], AT.Silu, bias=b_t[bnames[layer]][:, :1])
                    if layer == 0:
                        nc.sync.dma_start(out=h0T_d[:, w * _WIN:w * _WIN + ws], in_=xT[:])
                    if layer < 3:
                        hps = psB.tile([128, ws], f32, tag="mm128", name="hps2")
                        if layer == 2:
                            h0b = sp.tile([128, ws], f32, tag="h0tile", name="h0tile")
                            nc.sync.dma_start(out=h0b[:],
                                              in_=h0T_d[:, w * _WIN:w * _WIN + ws])
                        for st in range(nst):
                            if layer < 2:
                                nc.tensor.matmul(hps[:, st * 128:(st + 1) * 128],
                                                 lhsT=xT[:, st * 128:(st + 1) * 128],
                                                 rhs=wl_t[wnames[layer]][:],
                                                 start=(st == 0), stop=False,
                                                 skip_group_check=True)
                            else:
                                nc.tensor.matmul(hps[:, st * 128:(st + 1) * 128],
                                                 lhsT=xT[:, st * 128:(st + 1) * 128],
                                                 rhs=wl_t["w3ap"][:], start=(st == 0), stop=False,
                                                 skip_group_check=True)
                                nc.tensor.matmul(hps[:, st * 128:(st + 1) * 128],
                                                 lhsT=h0b[:, st * 128:(st + 1) * 128],
                                                 rhs=wl_t["w3bp"][:],
                                                 start=False, stop=False, skip_group_check=True)
                        hsout = sp.tile([128, nst, _F], bf16, tag="hsout", name="hsout")
                        nc.vector.tensor_tensor(
                            out=hsout[:], in0=hps[:].rearrange("p (s f) -> p s f", s=nst),
                            in1=self_h[:, 4 * w:4 * w + nst][:, :, None].to_broadcast([128, nst, _F]),
                            op=OP.mult)
                        nc.sync.dma_start(
                            out=_interleave(slice_d[layer + 1], w * _WIN, nst, _F),
                            in_=hsout[:])
                    else:
                        for st in range(ws // 128):
                            ops = psB.tile([128, 128], f32, tag="mm128", name="ops")
                            nc.tensor.transpose(ops[:], in_=xT[:, st * 128:(st + 1) * 128],
                                                identity=eye_t[:])
                            oc = sp.tile([128, 64], f32, tag="outt", name="outt")
                            nc.vector.tensor_copy(oc[:], ops[:, 0:64])
                            nc.sync.dma_start(
                                out=out_d.ap()[(w * _WIN + st * 128):(w * _WIN + st * 128 + 128), :],
                                in_=oc[:])

    nc.compile()
    return nc


def _get_compiled(inputs):
    in_maps, cpb, call_off, nchunks_w, chunk_meta, tot_chunks, tot_slots = _prep(inputs)
    key = cpb.tobytes()
    if key not in _compiled:
        _compiled[key] = _build(cpb, call_off, nchunks_w, chunk_meta, tot_chunks, tot_slots)
    return _compiled[key], in_maps


def _run(inputs, trace=False):
    _install_profile_shim()
    from concourse import bass_utils
    nc, in_maps = _get_compiled(inputs)
    res = bass_utils.run_bass_kernel_spmd(
        nc, in_maps, core_ids=list(range(_NCORES)), trace=trace)
    out = np.concatenate([res.results[k]["out"][:_SL] for k in range(_NCORES)], axis=0)
    return out[:_N].astype(np.float32), res.exec_time_ns


def kernel(**inputs):
    out, _ = _run(inputs, trace=False)
    return out



# revision 6
# speedup vs baseline: 1.0833x; 1.0833x over previous
"""Trainium2 Bass kernel for the 4-layer GCN diffusion denoiser (gnn_message_passing).

Strategy (8 NeuronCores, SPMD single program):
  - Nodes sharded 12500/core (padded to 12544 = 98*128). Edges routed to the core
    owning their dst node, grouped into 512-node windows.
  - Per layer, per-node features Hs = dinv * (X @ W) stored as fp16 [*, 128] rows,
    AllGather'ed across cores in FOUR quarter-slice chunks so the collective
    overlaps the previous layer's window compute. Gather bucket b = quarter b of
    every core's slice (view rows core*csize+local, int16-addressable).
  - Aggregation per window: descriptor-lean dma_gather streams (runs padded to
    x16 with dummy row-0 idxs, uniform across cores), segment-sum via PE matmuls
    against one-hot matrices built in a single fused DVE pass
    (iota==dstloc)*dinv_dst, self-loop via Hs_self x diag(dinv), Silu on ScalarE.
  - The next layer's H-matmul consumes the transposed activation tile directly.
"""

import math
import sys
import types

import numpy as np

_N, _E, _D, _G = 100000, 1000000, 64, 128
_NCORES = 8
_SL = _N // _NCORES          # 12500 real nodes per core
_SLP = 12544                 # padded per-core slice (98*128)
_NP = _SLP * _NCORES
_WIN = 512
_NWIN = (_SLP + _WIN - 1) // _WIN    # 25 windows (last is 256 nodes)
_NSUB = _SLP // 128          # 98 sub-tiles of 128 nodes
_F = 128
_PAD_DST = 1000.0
_CSTART = [0, 3072, 6144, 9216, 12544]          # quarter starts (local rows)
_CSIZE = [3072, 3072, 3072, 3328]
_NBUCK = 4
_MAX_CALL = 512

_compiled = {}


def _install_profile_shim():
    """Register the NTFF profile hook missing from this image's antenv."""
    try:
        import antenv
        from trn_agent_boot.trn_boot import _ntff_profile_via_ctypes
    except ImportError:
        return
    if "antenv.axon_hooks" in sys.modules:
        return
    mod = types.ModuleType("antenv.axon_hooks")
    hook = _ntff_profile_via_ctypes("/opt/axon/libaxon_pjrt.so")
    mod.get_axon_ntff_profile_hook = lambda: hook
    mod.set_axon_ntff_profile_hook = lambda h: None
    sys.modules["antenv.axon_hooks"] = mod
    antenv.axon_hooks = mod


def _ceil16(x):
    return (x + 15) // 16 * 16


class _Meta:
    """Shared (cross-core) gather stream structure."""
    pass


def _layout(runlen_max):
    """Build the shared call/chunk/pair layout from per-(w,b,st) max run lens."""
    m = _Meta()
    alloc = np.zeros((_NWIN, _NBUCK, 4), np.int64)
    for w in range(_NWIN):
        nst = min(_WIN, _SLP - w * _WIN) // 128
        for b in range(_NBUCK):
            for st in range(nst):
                r = int(runlen_max[w, b, st])
                alloc[w, b, st] = _ceil16(r) if r else 0
    m.alloc = alloc
    # run start offsets within the (w,b) stream
    m.run_off = np.zeros((_NWIN, _NBUCK, 4), np.int64)
    m.wb_size = np.zeros((_NWIN, _NBUCK), np.int64)
    for w in range(_NWIN):
        for b in range(_NBUCK):
            acc = 0
            for st in range(4):
                m.run_off[w, b, st] = acc
                acc += int(alloc[w, b, st])
            m.wb_size[w, b] = acc
    # global stream offset per (w,b)
    m.wb_base = np.zeros((_NWIN, _NBUCK), np.int64)
    acc = 0
    for w in range(_NWIN):
        for b in range(_NBUCK):
            m.wb_base[w, b] = acc
            acc += int(m.wb_size[w, b])
    m.s_total = acc
    # calls: split each (w,b) stream into pieces <= _MAX_CALL (x16 sizes)
    # each call owns cdiv(size,128) chunks in the window's chunk grid
    m.calls = []            # (w, b, stream_off, size, c0_window_local, nch)
    m.ncw = np.zeros(_NWIN, np.int64)
    m.pairs = []            # (w, c_local, st, slot_lo, slot_hi, stream_pos_lo)
    for w in range(_NWIN):
        cloc = 0
        for b in range(_NBUCK):
            size = int(m.wb_size[w, b])
            off = 0
            while off < size:
                csz = min(_MAX_CALL, size - off)
                nch = (csz + 127) // 128
                m.calls.append((w, b, int(m.wb_base[w, b] + off), csz, cloc, nch))
                # spans of this call's chunks against the run layout
                for k in range(nch):
                    lo = off + k * 128
                    hi = min(off + (k + 1) * 128, size)
                    for st in range(4):
                        ra, rs = int(m.alloc[w, b, st]), int(m.run_off[w, b, st])
                        if ra == 0:
                            continue
                        s_lo, s_hi = max(lo, rs), min(hi, rs + ra)
                        if s_lo < s_hi:
                            m.pairs.append((w, cloc + k, st, s_lo - lo, s_hi - lo,
                                            int(m.wb_base[w, b] + s_lo)))
                cloc += nch
                off += csz
        m.ncw[w] = cloc
    # pair index per window-local order
    m.pairs_by_w = [[] for _ in range(_NWIN)]
    for p in m.pairs:
        m.pairs_by_w[p[0]].append(p)
    m.npairs = len(m.pairs)
    return m


def _prep(inputs):
    """Host-side metadata build: edge bucketing, gather stream, one-hot scalars."""
    src = np.asarray(inputs["edge_index"][0], dtype=np.int64)
    dst = np.asarray(inputs["edge_index"][1], dtype=np.int64)
    deg = np.bincount(dst, minlength=_N).astype(np.float32) + 1.0
    dinv = (1.0 / np.sqrt(deg)).astype(np.float32)

    core = dst // _SL
    loc = dst % _SL
    w_of = loc // _WIN
    st_of = (loc % _WIN) // 128
    dstloc = (loc % 128).astype(np.float32)
    score = src // _SL
    sloc = src % _SL
    buck = np.digitize(sloc, _CSTART[1:4])
    csize_a = np.asarray(_CSIZE)
    cstart_a = np.asarray(_CSTART[:4])
    vrow = score * csize_a[buck] + (sloc - cstart_a[buck])
    dinv_d = dinv[dst]

    key = (((core * _NWIN + w_of) * _NBUCK + buck) * 4 + st_of)
    order = np.argsort(key, kind="stable")
    key_s = key[order]
    vrow_s = vrow[order].astype(np.int16)
    dstloc_s = dstloc[order]
    dinvd_s = dinv_d[order]

    nkeys = _NCORES * _NWIN * _NBUCK * 4
    counts = np.bincount(key_s, minlength=nkeys)
    runlen = counts.reshape(_NCORES, _NWIN, _NBUCK, 4)
    meta = _layout(runlen.max(axis=0))
    run_start = np.zeros(nkeys + 1, np.int64)
    np.cumsum(counts, out=run_start[1:])

    # stream position of every edge (per its own core's stream)
    rank = np.arange(len(key_s)) - run_start[key_s]
    wb_base_e = meta.wb_base[w_of[order], buck[order]]
    run_off_e = meta.run_off[w_of[order], buck[order], st_of[order]]
    pos = wb_base_e + run_off_e + rank          # stream slot per edge
    core_s = core[order]

    # pair lookup (w, c_local, st) -> window-local pair idx; and per-pair stream base
    pair_idx = {}
    pair_of_slot = {}
    for w in range(_NWIN):
        for i, p in enumerate(meta.pairs_by_w[w]):
            pair_idx[(w, p[1], p[2])] = i
    # global pair id (flat over windows, window-major)
    pair_gbase = np.zeros(_NWIN + 1, np.int64)
    for w in range(_NWIN):
        pair_gbase[w + 1] = pair_gbase[w] + len(meta.pairs_by_w[w])
    npairs_g = int(pair_gbase[_NWIN])

    # map every edge to (global pair, slot in chunk):
    # stream pos -> (w,b) known; slot_in_chunk and chunk derived from call split.
    # Rebuild per-(w,b) call boundaries for vectorized mapping.
    call_off_l = {}
    for (w, b, goff, csz, c0, nch) in meta.calls:
        call_off_l.setdefault((w, b), []).append((goff, csz, c0))
    # per-edge: find its call via offset within (w,b)
    gpair = np.zeros(len(pos), np.int64)
    slotc = np.zeros(len(pos), np.int64)
    wb_rel = pos - wb_base_e
    w_e, b_e, st_e = w_of[order], buck[order], st_of[order]
    for (w, b), calls in call_off_l.items():
        sel = (w_e == w) & (b_e == b)
        if not sel.any():
            continue
        rel = wb_rel[sel]
        st_sel = st_e[sel]
        gp = np.zeros(len(rel), np.int64)
        sc = np.zeros(len(rel), np.int64)
        wbb = int(meta.wb_base[w, b])
        for (goff, csz, c0) in calls:
            off = goff - wbb
            m2 = (rel >= off) & (rel < off + csz)
            if not m2.any():
                continue
            r2 = rel[m2] - off
            ch = r2 // 128
            sl = r2 % 128
            # pair id from (w, c0+ch, st)
            pid = np.array([pair_idx[(w, int(c0 + c), int(s))]
                            for c, s in zip(ch, st_sel[m2])], np.int64)
            gp[m2] = pair_gbase[w] + pid
            sc[m2] = sl
        gpair[sel] = gp
        slotc[sel] = sc

    is_norm = np.zeros(_N, bool)
    is_norm[np.asarray(inputs["train_norm"])] = True
    is_anm = np.zeros(_N, bool)
    is_anm[np.asarray(inputs["train_anm"])] = True

    noise = np.asarray(inputs["noise_x"], np.float32)
    t_val = float(np.asarray(inputs["t"]).reshape(-1)[0])
    half = _D // 2
    freqs = np.exp(
        np.arange(half, dtype=np.float32) * (-math.log(10000.0) / (half - 1))
    ).astype(np.float32)

    w_ = {m: np.asarray(inputs[m], np.float32) for m in
          ["w0", "b0", "w1", "b1", "w2", "b2", "w3", "b3",
           "time_w1", "time_b1", "time_w2", "time_b2", "label_emb"]}
    w1p = np.zeros((128, 128), np.float32); w1p[:, :64] = w_["w1"]
    w2p = np.zeros((128, 128), np.float32); w2p[:64, :] = w_["w2"]
    w3ap = np.zeros((128, 128), np.float32); w3ap[:, :64] = w_["w3"][:128]
    w3bp = np.zeros((128, 128), np.float32); w3bp[:, :64] = w_["w3"][128:]
    b1p = np.zeros((128, 1), np.float32); b1p[:64, 0] = w_["b1"]
    b3p = np.zeros((128, 1), np.float32); b3p[:64, 0] = w_["b3"]

    shared = {
        "w0": w_["w0"],
        "w1p": w1p, "w2p": w2p, "w3ap": w3ap, "w3bp": w3bp,
        "b0c": w_["b0"].reshape(128, 1).astype(np.float32),
        "b1c": b1p,
        "b2c": w_["b2"].reshape(128, 1).astype(np.float32),
        "b3c": b3p,
        "tw1": w_["time_w1"], "tw2": w_["time_w2"],
        "tb1": w_["time_b1"].reshape(64, 1).astype(np.float32),
        "tb2": w_["time_b2"].reshape(64, 1).astype(np.float32),
        "e0row": w_["label_emb"][0].reshape(1, 64).astype(np.float32),
        "e1row": w_["label_emb"][1].reshape(1, 64).astype(np.float32),
        "rsin": (np.mod(t_val * freqs + np.pi, 2 * np.pi) - np.pi).reshape(32, 1).astype(np.float32),
        "rcos": (np.mod(t_val * freqs + np.pi / 2 + np.pi, 2 * np.pi) - np.pi).reshape(32, 1).astype(np.float32),
        "ones1": np.ones((1, 128), np.float32),
    }

    in_maps = []
    for k in range(_NCORES):
        mine = core_s == k
        idx_stream = np.zeros(meta.s_total, np.int16)   # dummies -> view row 0
        idx_stream[pos[mine]] = vrow_s[mine]
        dstl_cols = np.full((npairs_g, 128), _PAD_DST, np.float32)
        dinv_cols = np.zeros((npairs_g, 128), np.float32)
        dstl_cols[gpair[mine], slotc[mine]] = dstloc_s[mine]
        dinv_cols[gpair[mine], slotc[mine]] = dinvd_s[mine]

        wrapped = np.tile(idx_stream.reshape(-1, 16).T, (8, 1))

        nodes = np.arange(_SLP) + k * _SL
        nodes_c = np.minimum(nodes, _N - 1)
        sd = dinv[nodes_c].copy()
        sd[np.arange(_SLP) >= _SL] = 1.0
        selfdinv = sd.reshape(_NSUB, 128).T.copy()

        s_n = np.zeros(_SLP, np.float32)
        s_a = np.zeros(_SLP, np.float32)
        real = np.arange(_SLP) < _SL
        s_n[real] = is_norm[nodes[real]]
        s_a[real] = is_anm[nodes[real]] & ~is_norm[nodes[real]]

        nz = np.zeros((_SLP, _D), np.float32)
        nz[:_SL] = noise[k * _SL:(k + 1) * _SL]

        m = dict(shared)
        m.update({
            "midx": wrapped,
            "mdstl": np.ascontiguousarray(dstl_cols.T),
            "mdinvd": np.ascontiguousarray(dinv_cols.T),
            "mself": selfdinv,
            "ms0": s_n.reshape(_NSUB, 128).T.copy(),
            "ms1": s_a.reshape(_NSUB, 128).T.copy(),
            "noise": nz,
        })
        in_maps.append(m)

    return in_maps, meta, npairs_g


def _build(meta, npairs_g):
    import concourse.bass as bass
    import concourse.bacc as bacc
    import concourse.tile as tile
    from concourse import mybir
    from concourse.masks import make_identity

    f32 = mybir.dt.float32
    f16 = mybir.dt.float16
    AT = mybir.ActivationFunctionType
    OP = mybir.AluOpType

    nc = bacc.Bacc("TRN2", target_bir_lowering=False, debug=False,
                   num_devices=_NCORES, dynamic_dma_scratch_size=32768,
                   num_swdge_queues=4)

    din = {}
    def dt_in(name, shape, dt):
        din[name] = nc.dram_tensor(name, list(shape), dt, kind="ExternalInput")
        return din[name]

    dt_in("noise", (_SLP, _D), f32)
    dt_in("midx", (128, meta.s_total // 16), mybir.dt.int16)
    dt_in("mdstl", (128, npairs_g), f32)
    dt_in("mdinvd", (128, npairs_g), f32)
    dt_in("mself", (128, _NSUB), f32)
    dt_in("ms0", (128, _NSUB), f32)
    dt_in("ms1", (128, _NSUB), f32)
    dt_in("w0", (64, 128), f32)
    for nm in ["w1p", "w2p", "w3ap", "w3bp"]:
        dt_in(nm, (128, 128), f32)
    for nm in ["b0c", "b1c", "b2c", "b3c"]:
        dt_in(nm, (128, 1), f32)
    dt_in("tw1", (64, 64), f32)
    dt_in("tw2", (64, 64), f32)
    dt_in("tb1", (64, 1), f32)
    dt_in("tb2", (64, 1), f32)
    dt_in("e0row", (1, 64), f32)
    dt_in("e1row", (1, 64), f32)
    dt_in("rsin", (32, 1), f32)
    dt_in("rcos", (32, 1), f32)
    dt_in("ones1", (1, 128), f32)
    out_d = nc.dram_tensor("out", [_SLP, 64], f32, kind="ExternalOutput")

    wnames = ["w1p", "w2p", None]
    bnames = ["b0c", "b1c", "b2c", "b3c"]
    ncw_max = int(meta.ncw.max())
    npw_max = max(len(meta.pairs_by_w[w]) for w in range(_NWIN))
    pair_gbase = [0]
    for w in range(_NWIN):
        pair_gbase.append(pair_gbase[-1] + len(meta.pairs_by_w[w]))

    def _interleave(dram_tile, row0, nst, F):
        ap = dram_tile[:]
        return bass.AP(ap.tensor, ap.offset + row0 * F,
                       [[F, 128], [128 * F, nst], [1, F]])

    qctr = [0]
    def next_q():
        q = qctr[0] % 4
        qctr[0] += 1
        return q

    # calls grouped per window for issue order
    calls_by_w = [[] for _ in range(_NWIN)]
    for c in meta.calls:
        calls_by_w[c[0]].append(c)

    with tile.TileContext(nc) as tc:
        with tc.tile_pool(name="consts", bufs=1) as cp, \
             tc.tile_pool(name="meta", bufs=1) as mp, \
             tc.tile_pool(name="dram", bufs=1, space="DRAM") as dram, \
             tc.tile_pool(name="g", bufs=3) as gp, \
             tc.tile_pool(name="oh", bufs=2) as ohp, \
             tc.tile_pool(name="small", bufs=4) as sp, \
             tc.tile_pool(name="xt", bufs=2) as xtp, \
             tc.tile_pool(name="psA", bufs=2, space="PSUM") as psA, \
             tc.tile_pool(name="psB", bufs=3, space="PSUM") as psB, \
             tc.tile_pool(name="psC", bufs=2, space="PSUM") as psC:

            def load(name, shape, dt, pool=cp):
                t = pool.tile(list(shape), dt, tag=name, name=name)
                nc.sync.dma_start(out=t[:], in_=din[name].ap())
                return t

            idx_t = load("midx", (128, meta.s_total // 16), mybir.dt.int16, mp)
            dstl_t = load("mdstl", (128, npairs_g), f32, mp)
            dinvd_t = load("mdinvd", (128, npairs_g), f32, mp)
            self_t = load("mself", (128, _NSUB), f32)
            s0_t = load("ms0", (128, _NSUB), f32)
            s1_t = load("ms1", (128, _NSUB), f32)
            w0_t = load("w0", (64, 128), f32)
            wl_t = {nm: load(nm, (128, 128), f32) for nm in ["w1p", "w2p", "w3ap", "w3bp"]}
            b_t = {nm: load(nm, (128, 1), f32) for nm in bnames}
            tw1_t = load("tw1", (64, 64), f32)
            tw2_t = load("tw2", (64, 64), f32)
            tb1_t = load("tb1", (64, 1), f32)
            tb2_t = load("tb2", (64, 1), f32)
            e0_t = load("e0row", (1, 64), f32)
            e1_t = load("e1row", (1, 64), f32)
            rsin_t = load("rsin", (32, 1), f32)
            rcos_t = load("rcos", (32, 1), f32)
            ones1_t = load("ones1", (1, 128), f32)

            iota_i = cp.tile([128, 128], mybir.dt.int32, tag="iotai", name="iotai")
            nc.gpsimd.iota(iota_i[:], pattern=[[1, 128]], base=0, channel_multiplier=0)
            iota_b = cp.tile([128, 128], f32, tag="iotab", name="iotab")
            nc.vector.tensor_copy(iota_b[:], iota_i[:])
            self_h = cp.tile([128, _NSUB], f16, tag="selfh", name="selfh")
            nc.vector.tensor_copy(self_h[:], self_t[:])
            eye_t = cp.tile([128, 128], f32, tag="eye", name="eye")
            make_identity(nc, eye_t[:])
            eye_h = cp.tile([128, 128], f16, tag="eyeh", name="eyeh")
            nc.vector.tensor_copy(eye_h[:], eye_t[:])

            # zero the gather buffers once (stale tails feed 0-masked matmuls)
            for _z in range(3):
                gz = gp.tile([128, ncw_max, _F], f16, tag="g", name="g")
                nc.vector.memset(gz[:], 0)

            # ---- DRAM working buffers ----
            slice_d = [[dram.tile([_CSIZE[c], _F], f16, tag=f"sl{l}_{c}",
                                  name=f"sl{l}_{c}") for c in range(4)]
                       for l in range(4)]
            full_d = [[dram.tile([_CSIZE[c] * _NCORES, _F], f16, tag=f"fu{l}_{c}",
                                 name=f"fu{l}_{c}") for c in range(4)]
                      for l in range(4)]
            h0T_d = dram.tile([128, _SLP], f32, tag="h0T", name="h0T")

            def ag(l, c):
                nc.gpsimd.collective_compute(
                    "AllGather", mybir.AluOpType.bypass,
                    replica_groups=[list(range(_NCORES))],
                    ins=[slice_d[l][c].opt()], outs=[full_d[l][c].opt()],
                )

            # window w rows [w*512, w*512+ws) -> quarter helpers
            def quarter_of_row(r):
                for c in range(4):
                    if r < _CSTART[c + 1]:
                        return c
                raise AssertionError

            def write_slice(l, row0, nst, src_ap):
                # rows [row0, row0+nst*128) always lie inside one quarter
                c = quarter_of_row(row0)
                assert row0 + nst * 128 <= _CSTART[c + 1]
                nc.sync.dma_start(
                    out=_interleave(slice_d[l][c], row0 - _CSTART[c], nst, _F),
                    in_=src_ap)

            def read_slice(l, row0, nst, dst_ap):
                c = quarter_of_row(row0)
                nc.sync.dma_start(
                    out=dst_ap,
                    in_=_interleave(slice_d[l][c], row0 - _CSTART[c], nst, _F))

            # ---- timestep embedding ----
            sc_t = sp.tile([64, 1], f32, tag="tsc", name="tsc")
            nc.scalar.activation(sc_t[:32, :], rsin_t[:], AT.Sin)
            nc.scalar.activation(sc_t[32:64, :], rcos_t[:], AT.Sin)
            h1ps = psC.tile([64, 1], f32, tag="temb", name="h1ps")
            nc.tensor.matmul(h1ps[:], lhsT=tw1_t[:], rhs=sc_t[:], start=True, stop=True)
            h1_t = sp.tile([64, 1], f32, tag="th1", name="th1")
            nc.scalar.activation(h1_t[:], h1ps[:], AT.Silu, bias=tb1_t[:, :1])
            t2ps = psC.tile([64, 1], f32, tag="temb", name="t2ps")
            nc.tensor.matmul(t2ps[:], lhsT=tw2_t[:], rhs=h1_t[:], start=True, stop=True)
            tembT = sp.tile([64, 1], f32, tag="tembT", name="tembT")
            nc.vector.tensor_scalar(out=tembT[:], in0=t2ps[:], scalar1=tb2_t[:, :1],
                                    scalar2=None, op0=OP.add)
            trow_ps = psC.tile([1, 64], f32, tag="temb", name="trow_ps")
            nc.tensor.transpose(trow_ps[:], in_=tembT[:], identity=eye_t[:64, :64])
            trow_t = sp.tile([1, 64], f32, tag="trowS", name="trowS")
            nc.vector.tensor_copy(trow_t[:], trow_ps[:])
            rows_ps = psC.tile([128, 192], f32, tag="temb", name="rows_ps")
            nc.tensor.matmul(rows_ps[:, 0:64], lhsT=ones1_t[:], rhs=trow_t[:],
                             start=True, stop=True, skip_group_check=True)
            nc.tensor.matmul(rows_ps[:, 64:128], lhsT=ones1_t[:], rhs=e0_t[:],
                             start=True, stop=True, skip_group_check=True)
            nc.tensor.matmul(rows_ps[:, 128:192], lhsT=ones1_t[:], rhs=e1_t[:],
                             start=True, stop=True, skip_group_check=True)
            addrows = cp.tile([128, 192], f32, tag="addrows", name="addrows")
            nc.vector.tensor_copy(addrows[:], rows_ps[:])

            # ---- x0 phase: Hs0 = dinv * ((noise + temb + lab) @ w0), batched x4 ----
            groups = [(g0 * 4, min(4, _NSUB - g0 * 4)) for g0 in range((_NSUB + 3) // 4)]
            ag0_done = 0
            for (st0, ng) in groups:
                nz = sp.tile([128, ng, 64], f32, tag="nz", name="nz")
                nap = din["noise"].ap()
                nc.sync.dma_start(
                    out=nz[:],
                    in_=bass.AP(nap.tensor, nap.offset + st0 * 128 * _D,
                                [[_D, 128], [128 * _D, ng], [1, _D]]))
                x0 = sp.tile([128, ng, 64], f32, tag="x0", name="x0")
                tr_b = bass.AP(addrows[:].tensor, addrows[:].offset,
                               [list(addrows[:].ap[0]), [0, ng], [1, 64]])
                nc.vector.tensor_tensor(out=x0[:], in0=nz[:], in1=tr_b, op=OP.add)
                lab = sp.tile([128, ng, 64], f32, tag="lab", name="lab")
                e0_b = bass.AP(addrows[:].tensor, addrows[:].offset + 64,
                               [list(addrows[:].ap[0]), [0, ng], [1, 64]])
                nc.vector.tensor_tensor(
                    out=lab[:], in0=e0_b,
                    in1=s0_t[:, st0:st0 + ng][:, :, None].to_broadcast([128, ng, 64]),
                    op=OP.mult)
                nc.vector.tensor_add(x0[:], x0[:], lab[:])
                e1_b = bass.AP(addrows[:].tensor, addrows[:].offset + 128,
                               [list(addrows[:].ap[0]), [0, ng], [1, 64]])
                nc.vector.tensor_tensor(
                    out=lab[:], in0=e1_b,
                    in1=s1_t[:, st0:st0 + ng][:, :, None].to_broadcast([128, ng, 64]),
                    op=OP.mult)
                nc.vector.tensor_add(x0[:], x0[:], lab[:])
                hs0 = sp.tile([128, ng, _F], f16, tag="hsout", name="hsout")
                for j in range(ng):
                    x0T_ps = psB.tile([64, 128], f32, tag="mm128", name="x0T_ps")
                    nc.tensor.transpose(x0T_ps[:], in_=x0[:, j, :], identity=eye_t[:])
                    x0T = sp.tile([64, 128], f32, tag="x0Ts", name="x0Ts")
                    nc.vector.tensor_copy(x0T[:], x0T_ps[:])
                    hps = psB.tile([128, 128], f32, tag="mm128", name="hps")
                    nc.tensor.matmul(hps[:], lhsT=x0T[:], rhs=w0_t[:], start=True, stop=True)
                    nc.vector.tensor_scalar(
                        out=hs0[:, j, :], in0=hps[:],
                        scalar1=self_t[:, st0 + j:st0 + j + 1], scalar2=None, op0=OP.mult)
                write_slice(0, st0 * 128, ng, hs0[:])
                # chunked AG0 as quarters complete (quarter ends at subtile 24/48/72/98)
                done_rows = (st0 + ng) * 128
                while ag0_done < 4 and done_rows >= _CSTART[ag0_done + 1]:
                    ag(0, ag0_done)
                    ag0_done += 1

            # ---- layers ----
            for layer in range(4):
                ag_next = 0
                for w in range(_NWIN):
                    ws = min(_WIN, _SLP - w * _WIN)
                    nst = ws // 128
                    ncw = int(meta.ncw[w])
                    npw = len(meta.pairs_by_w[w])
                    g = gp.tile([128, ncw_max, _F], f16, tag="g", name="g")
                    for (_, b, goff, csz, c0, nch) in calls_by_w[w]:
                        nc.gpsimd.dma_gather(
                            out_ap=g[:, c0:c0 + nch, :],
                            in_ap=full_d[layer][b][:],
                            idxs_ap=idx_t[:, goff // 16: goff // 16 + csz // 16],
                            num_idxs=csz, num_idxs_reg=csz, elem_size=_F,
                            queue_num=next_q(), single_packet=False,
                        )
                    # launch next layer's AG chunks once prior windows wrote them
                    if layer < 3 and ag_next < 4 and w * _WIN >= _CSTART[ag_next + 1] + 1024:
                        ag(layer + 1, ag_next)
                        ag_next += 1
                    # fused one-hots: (iota == dstl) * dinv_dst, one DVE op per pair
                    oh = ohp.tile([128, npw_max, 128], f16, tag="oh", name="oh")
                    for i in range(npw):
                        gi = pair_gbase[w] + i
                        nc.vector.tensor_scalar(
                            out=oh[:, i, :], in0=iota_b[:],
                            scalar1=dstl_t[:, gi:gi + 1], scalar2=dinvd_t[:, gi:gi + 1],
                            op0=OP.is_equal, op1=OP.mult)
                    agg = psA.tile([128, ws], f32, tag="agg", name="agg")
                    for i, p in enumerate(meta.pairs_by_w[w]):
                        _, c_local, st_c, _, _, _ = p
                        nc.tensor.matmul(agg[:, st_c * 128:(st_c + 1) * 128],
                                         lhsT=g[:, c_local, :], rhs=oh[:, i, :],
                                         start=(i == 0), stop=False,
                                         skip_group_check=True)
                    # self-loop terms
                    hsb = sp.tile([128, nst, _F], f16, tag="hself", name="hself")
                    read_slice(layer, w * _WIN, nst, hsb[:])
                    srhs = sp.tile([128, nst, 128], f16, tag="srhs", name="srhs")
                    eye_rep = bass.AP(eye_h[:].tensor, eye_h[:].offset,
                                      [list(eye_h[:].ap[0]), [0, nst], [1, 128]])
                    nc.vector.tensor_tensor(
                        out=srhs[:], in0=eye_rep,
                        in1=self_h[:, 4 * w:4 * w + nst][:, :, None].to_broadcast([128, nst, 128]),
                        op=OP.mult)
                    for st in range(nst):
                        nc.tensor.matmul(agg[:, st * 128:(st + 1) * 128],
                                         lhsT=hsb[:, st, :], rhs=srhs[:, st, :],
                                         start=(npw == 0 and st == 0),
                                         stop=(st == nst - 1),
                                         skip_group_check=True)
                    xT = xtp.tile([128, ws], f32, tag="xT", name="xT")
                    nc.scalar.activation(xT[:], agg[:], AT.Silu, bias=b_t[bnames[layer]][:, :1])
                    if layer == 0:
                        nc.sync.dma_start(out=h0T_d[:, w * _WIN:w * _WIN + ws], in_=xT[:])
                    if layer < 3:
                        hps = psB.tile([128, ws], f32, tag="mm128", name="hps2")
                        if layer == 2:
                            h0b = sp.tile([128, ws], f32, tag="h0tile", name="h0tile")
                            nc.sync.dma_start(out=h0b[:],
                                              in_=h0T_d[:, w * _WIN:w * _WIN + ws])
                        for st in range(nst):
                            if layer < 2:
                                nc.tensor.matmul(hps[:, st * 128:(st + 1) * 128],
                                                 lhsT=xT[:, st * 128:(st + 1) * 128],
                                                 rhs=wl_t[wnames[layer]][:],
                                                 start=(st == 0), stop=False,
                                                 skip_group_check=True)
                            else:
                                nc.tensor.matmul(hps[:, st * 128:(st + 1) * 128],
                                                 lhsT=xT[:, st * 128:(st + 1) * 128],
                                                 rhs=wl_t["w3ap"][:], start=(st == 0), stop=False,
                                                 skip_group_check=True)
                                nc.tensor.matmul(hps[:, st * 128:(st + 1) * 128],
                                                 lhsT=h0b[:, st * 128:(st + 1) * 128],
                                                 rhs=wl_t["w3bp"][:],
                                                 start=False, stop=False, skip_group_check=True)
                        hsout = sp.tile([128, nst, _F], f16, tag="hsout", name="hsout")
                        nc.vector.tensor_tensor(
                            out=hsout[:], in0=hps[:].rearrange("p (s f) -> p s f", s=nst),
                            in1=self_h[:, 4 * w:4 * w + nst][:, :, None].to_broadcast([128, nst, _F]),
                            op=OP.mult)
                        write_slice(layer + 1, w * _WIN, nst, hsout[:])
                    else:
                        for st in range(ws // 128):
                            ops = psB.tile([128, 128], f32, tag="mm128", name="ops")
                            nc.tensor.transpose(ops[:], in_=xT[:, st * 128:(st + 1) * 128],
                                                identity=eye_t[:])
                            oc = sp.tile([128, 64], f32, tag="outt", name="outt")
                            nc.vector.tensor_copy(oc[:], ops[:, 0:64])
                            nc.sync.dma_start(
                                out=out_d.ap()[(w * _WIN + st * 128):(w * _WIN + st * 128 + 128), :],
                                in_=oc[:])
                # tail AG chunks for the next layer
                if layer < 3:
                    while ag_next < 4:
                        ag(layer + 1, ag_next)
                        ag_next += 1

    nc.compile()
    return nc


def _get_compiled(inputs):
    in_maps, meta, npairs_g = _prep(inputs)
    key = meta.alloc.tobytes()
    if key not in _compiled:
        _compiled[key] = _build(meta, npairs_g)
    return _compiled[key], in_maps


def _run(inputs, trace=False):
    _install_profile_shim()
    from concourse import bass_utils
    nc, in_maps = _get_compiled(inputs)
    res = bass_utils.run_bass_kernel_spmd(
        nc, in_maps, core_ids=list(range(_NCORES)), trace=trace)
    out = np.concatenate([res.results[k]["out"][:_SL] for k in range(_NCORES)], axis=0)
    return out[:_N].astype(np.float32), res.exec_time_ns


def kernel(**inputs):
    out, _ = _run(inputs, trace=False)
    return out


# revision 8
# speedup vs baseline: 1.2144x; 1.1210x over previous
"""Trainium2 Bass kernel for the 4-layer GCN diffusion denoiser (gnn_message_passing).

Strategy (8 NeuronCores, SPMD single program):
  - Nodes sharded 12500/core (padded to 12544 = 98*128). Edges routed to the core
    owning their dst node, grouped into 512-node windows.
  - Per layer, per-node features Hs = dinv * (X @ W) stored as fp16 [*, 128] rows,
    AllGather'ed across cores in FOUR quarter-slice chunks so the collective
    overlaps the previous layer's window compute. Gather bucket b = quarter b of
    every core's slice (view rows core*csize+local, int16-addressable).
  - Aggregation per window: descriptor-lean dma_gather streams (runs padded to
    x16 with dummy row-0 idxs, uniform across cores), segment-sum via PE matmuls
    against one-hot matrices built in a single fused DVE pass
    (iota==dstloc)*dinv_dst, self-loop via Hs_self x diag(dinv), Silu on ScalarE.
  - The next layer's H-matmul consumes the transposed activation tile directly.
"""

import math
import sys
import types

import numpy as np

_N, _E, _D, _G = 100000, 1000000, 64, 128
_NCORES = 8
_SL = _N // _NCORES          # 12500 real nodes per core
_SLP = 12544                 # padded per-core slice (98*128)
_NP = _SLP * _NCORES
_WIN = 512
_NWIN = (_SLP + _WIN - 1) // _WIN    # 25 windows (last is 256 nodes)
_NSUB = _SLP // 128          # 98 sub-tiles of 128 nodes
_F = 128
_PAD_DST = 1000.0
_CSTART = [0, 3072, 6144, 9216, 12544]          # quarter starts (local rows)
_CSIZE = [3072, 3072, 3072, 3328]
_NBUCK = 4
_MAX_CALL = 512

_compiled = {}


def _install_profile_shim():
    """Register the NTFF profile hook missing from this image's antenv."""
    try:
        import antenv
        from trn_agent_boot.trn_boot import _ntff_profile_via_ctypes
    except ImportError:
        return
    if "antenv.axon_hooks" in sys.modules:
        return
    mod = types.ModuleType("antenv.axon_hooks")
    hook = _ntff_profile_via_ctypes("/opt/axon/libaxon_pjrt.so")
    mod.get_axon_ntff_profile_hook = lambda: hook
    mod.set_axon_ntff_profile_hook = lambda h: None
    sys.modules["antenv.axon_hooks"] = mod
    antenv.axon_hooks = mod


def _ceil16(x):
    return (x + 15) // 16 * 16


class _Meta:
    """Shared (cross-core) gather stream structure."""
    pass


def _layout(runlen_max):
    """Build the shared call/chunk/pair layout from per-(w,b,st) max run lens."""
    m = _Meta()
    alloc = np.zeros((_NWIN, _NBUCK, 4), np.int64)
    for w in range(_NWIN):
        nst = min(_WIN, _SLP - w * _WIN) // 128
        for b in range(_NBUCK):
            for st in range(nst):
                r = int(runlen_max[w, b, st])
                alloc[w, b, st] = _ceil16(r) if r else 0
    m.alloc = alloc
    # run start offsets within the (w,b) stream
    m.run_off = np.zeros((_NWIN, _NBUCK, 4), np.int64)
    m.wb_size = np.zeros((_NWIN, _NBUCK), np.int64)
    for w in range(_NWIN):
        for b in range(_NBUCK):
            acc = 0
            for st in range(4):
                m.run_off[w, b, st] = acc
                acc += int(alloc[w, b, st])
            m.wb_size[w, b] = acc
    # global stream offset per (w,b)
    m.wb_base = np.zeros((_NWIN, _NBUCK), np.int64)
    acc = 0
    for w in range(_NWIN):
        for b in range(_NBUCK):
            m.wb_base[w, b] = acc
            acc += int(m.wb_size[w, b])
    m.s_total = acc
    # calls: split each (w,b) stream into pieces <= _MAX_CALL (x16 sizes)
    # each call owns cdiv(size,128) chunks in the window's chunk grid
    m.calls = []            # (w, b, stream_off, size, c0_window_local, nch)
    m.ncw = np.zeros(_NWIN, np.int64)
    m.pairs = []            # (w, c_local, st, slot_lo, slot_hi, stream_pos_lo)
    for w in range(_NWIN):
        cloc = 0
        for b in range(_NBUCK):
            size = int(m.wb_size[w, b])
            off = 0
            while off < size:
                csz = min(_MAX_CALL, size - off)
                nch = (csz + 127) // 128
                m.calls.append((w, b, int(m.wb_base[w, b] + off), csz, cloc, nch))
                # spans of this call's chunks against the run layout
                for k in range(nch):
                    lo = off + k * 128
                    hi = min(off + (k + 1) * 128, size)
                    for st in range(4):
                        ra, rs = int(m.alloc[w, b, st]), int(m.run_off[w, b, st])
                        if ra == 0:
                            continue
                        s_lo, s_hi = max(lo, rs), min(hi, rs + ra)
                        if s_lo < s_hi:
                            m.pairs.append((w, cloc + k, st, s_lo - lo, s_hi - lo,
                                            int(m.wb_base[w, b] + s_lo)))
                cloc += nch
                off += csz
        m.ncw[w] = cloc
    # pair index per window-local order
    m.pairs_by_w = [[] for _ in range(_NWIN)]
    for p in m.pairs:
        m.pairs_by_w[p[0]].append(p)
    m.npairs = len(m.pairs)
    return m


def _prep(inputs):
    """Host-side metadata build: edge bucketing, gather stream, one-hot scalars."""
    src = np.asarray(inputs["edge_index"][0], dtype=np.int64)
    dst = np.asarray(inputs["edge_index"][1], dtype=np.int64)
    deg = np.bincount(dst, minlength=_N).astype(np.float32) + 1.0
    dinv = (1.0 / np.sqrt(deg)).astype(np.float32)

    core = dst // _SL
    loc = dst % _SL
    w_of = loc // _WIN
    st_of = (loc % _WIN) // 128
    dstloc = (loc % 128).astype(np.float32)
    score = src // _SL
    sloc = src % _SL
    buck = np.digitize(sloc, _CSTART[1:4])
    csize_a = np.asarray(_CSIZE)
    cstart_a = np.asarray(_CSTART[:4])
    vrow = score * csize_a[buck] + (sloc - cstart_a[buck])
    dinv_d = dinv[dst]

    key = (((core * _NWIN + w_of) * _NBUCK + buck) * 4 + st_of)
    order = np.argsort(key, kind="stable")
    key_s = key[order]
    vrow_s = vrow[order].astype(np.int16)
    dstloc_s = dstloc[order]
    dinvd_s = dinv_d[order]

    nkeys = _NCORES * _NWIN * _NBUCK * 4
    counts = np.bincount(key_s, minlength=nkeys)
    runlen = counts.reshape(_NCORES, _NWIN, _NBUCK, 4)
    meta = _layout(runlen.max(axis=0))
    run_start = np.zeros(nkeys + 1, np.int64)
    np.cumsum(counts, out=run_start[1:])

    # stream position of every edge (per its own core's stream)
    rank = np.arange(len(key_s)) - run_start[key_s]
    wb_base_e = meta.wb_base[w_of[order], buck[order]]
    run_off_e = meta.run_off[w_of[order], buck[order], st_of[order]]
    pos = wb_base_e + run_off_e + rank          # stream slot per edge
    core_s = core[order]

    # pair lookup (w, c_local, st) -> window-local pair idx; and per-pair stream base
    pair_idx = {}
    pair_of_slot = {}
    for w in range(_NWIN):
        for i, p in enumerate(meta.pairs_by_w[w]):
            pair_idx[(w, p[1], p[2])] = i
    # global pair id (flat over windows, window-major)
    pair_gbase = np.zeros(_NWIN + 1, np.int64)
    for w in range(_NWIN):
        pair_gbase[w + 1] = pair_gbase[w] + len(meta.pairs_by_w[w])
    npairs_g = int(pair_gbase[_NWIN])

    # map every edge to (global pair, slot in chunk):
    # stream pos -> (w,b) known; slot_in_chunk and chunk derived from call split.
    # Rebuild per-(w,b) call boundaries for vectorized mapping.
    call_off_l = {}
    for (w, b, goff, csz, c0, nch) in meta.calls:
        call_off_l.setdefault((w, b), []).append((goff, csz, c0))
    # per-edge: find its call via offset within (w,b)
    gpair = np.zeros(len(pos), np.int64)
    slotc = np.zeros(len(pos), np.int64)
    wb_rel = pos - wb_base_e
    w_e, b_e, st_e = w_of[order], buck[order], st_of[order]
    for (w, b), calls in call_off_l.items():
        sel = (w_e == w) & (b_e == b)
        if not sel.any():
            continue
        rel = wb_rel[sel]
        st_sel = st_e[sel]
        gp = np.zeros(len(rel), np.int64)
        sc = np.zeros(len(rel), np.int64)
        wbb = int(meta.wb_base[w, b])
        for (goff, csz, c0) in calls:
            off = goff - wbb
            m2 = (rel >= off) & (rel < off + csz)
            if not m2.any():
                continue
            r2 = rel[m2] - off
            ch = r2 // 128
            sl = r2 % 128
            # pair id from (w, c0+ch, st)
            pid = np.array([pair_idx[(w, int(c0 + c), int(s))]
                            for c, s in zip(ch, st_sel[m2])], np.int64)
            gp[m2] = pair_gbase[w] + pid
            sc[m2] = sl
        gpair[sel] = gp
        slotc[sel] = sc

    is_norm = np.zeros(_N, bool)
    is_norm[np.asarray(inputs["train_norm"])] = True
    is_anm = np.zeros(_N, bool)
    is_anm[np.asarray(inputs["train_anm"])] = True

    noise = np.asarray(inputs["noise_x"], np.float32)
    t_val = float(np.asarray(inputs["t"]).reshape(-1)[0])
    half = _D // 2
    freqs = np.exp(
        np.arange(half, dtype=np.float32) * (-math.log(10000.0) / (half - 1))
    ).astype(np.float32)

    w_ = {m: np.asarray(inputs[m], np.float32) for m in
          ["w0", "b0", "w1", "b1", "w2", "b2", "w3", "b3",
           "time_w1", "time_b1", "time_w2", "time_b2", "label_emb"]}
    w1p = np.zeros((128, 128), np.float32); w1p[:, :64] = w_["w1"]
    w2p = np.zeros((128, 128), np.float32); w2p[:64, :] = w_["w2"]
    w3ap = np.zeros((128, 128), np.float32); w3ap[:, :64] = w_["w3"][:128]
    w3bp = np.zeros((128, 128), np.float32); w3bp[:, :64] = w_["w3"][128:]
    b1p = np.zeros((128, 1), np.float32); b1p[:64, 0] = w_["b1"]
    b3p = np.zeros((128, 1), np.float32); b3p[:64, 0] = w_["b3"]

    shared = {
        "w0": w_["w0"],
        "w1p": w1p, "w2p": w2p, "w3ap": w3ap, "w3bp": w3bp,
        "b0c": w_["b0"].reshape(128, 1).astype(np.float32),
        "b1c": b1p,
        "b2c": w_["b2"].reshape(128, 1).astype(np.float32),
        "b3c": b3p,
        "tw1": w_["time_w1"], "tw2": w_["time_w2"],
        "tb1": w_["time_b1"].reshape(64, 1).astype(np.float32),
        "tb2": w_["time_b2"].reshape(64, 1).astype(np.float32),
        "e0row": w_["label_emb"][0].reshape(1, 64).astype(np.float32),
        "e1row": w_["label_emb"][1].reshape(1, 64).astype(np.float32),
        "rsin": (np.mod(t_val * freqs + np.pi, 2 * np.pi) - np.pi).reshape(32, 1).astype(np.float32),
        "rcos": (np.mod(t_val * freqs + np.pi / 2 + np.pi, 2 * np.pi) - np.pi).reshape(32, 1).astype(np.float32),
        "ones1": np.ones((1, 128), np.float32),
    }

    in_maps = []
    for k in range(_NCORES):
        mine = core_s == k
        idx_stream = np.zeros(meta.s_total, np.int16)   # dummies -> view row 0
        idx_stream[pos[mine]] = vrow_s[mine]
        ohs = np.zeros((128, npairs_g, 128), np.float16)
        ohs[slotc[mine], gpair[mine], dstloc_s[mine].astype(np.int64)] = \
            dinvd_s[mine].astype(np.float16)

        wrapped = np.tile(idx_stream.reshape(-1, 16).T, (8, 1))

        nodes = np.arange(_SLP) + k * _SL
        nodes_c = np.minimum(nodes, _N - 1)
        sd = dinv[nodes_c].copy()
        sd[np.arange(_SLP) >= _SL] = 1.0
        selfdinv = sd.reshape(_NSUB, 128).T.copy()

        s_n = np.zeros(_SLP, np.float32)
        s_a = np.zeros(_SLP, np.float32)
        real = np.arange(_SLP) < _SL
        s_n[real] = is_norm[nodes[real]]
        s_a[real] = is_anm[nodes[real]] & ~is_norm[nodes[real]]

        nz = np.zeros((_SLP, _D), np.float32)
        nz[:_SL] = noise[k * _SL:(k + 1) * _SL]

        # srhs[p, s, j] = eye[p, j] * selfdinv_of_node(s*128+p)
        srhs = np.zeros((128, _NSUB, 128), np.float16)
        sd = selfdinv  # [128, _NSUB] partition-major
        for ss in range(_NSUB):
            np.fill_diagonal(srhs[:, ss, :], sd[:, ss])

        m = dict(shared)
        m.update({
            "midx": wrapped,
            "moh": ohs.reshape(128, npairs_g * 128),
            "msrhs": srhs.reshape(128, _NSUB * 128),
            "mself": selfdinv,
            "ms0": s_n.reshape(_NSUB, 128).T.copy(),
            "ms1": s_a.reshape(_NSUB, 128).T.copy(),
            "noise": nz,
        })
        in_maps.append(m)

    return in_maps, meta, npairs_g


def _build(meta, npairs_g):
    import concourse.bass as bass
    import concourse.bacc as bacc
    import concourse.tile as tile
    from concourse import mybir
    from concourse.masks import make_identity

    f32 = mybir.dt.float32
    f16 = mybir.dt.float16
    AT = mybir.ActivationFunctionType
    OP = mybir.AluOpType

    nc = bacc.Bacc("TRN2", target_bir_lowering=False, debug=False,
                   num_devices=_NCORES, dynamic_dma_scratch_size=32768,
                   num_swdge_queues=4)

    din = {}
    def dt_in(name, shape, dt):
        din[name] = nc.dram_tensor(name, list(shape), dt, kind="ExternalInput")
        return din[name]

    dt_in("noise", (_SLP, _D), f32)
    dt_in("midx", (128, meta.s_total // 16), mybir.dt.int16)
    dt_in("moh", (128, npairs_g * 128), f16)
    dt_in("msrhs", (128, _NSUB * 128), f16)
    dt_in("mself", (128, _NSUB), f32)
    dt_in("ms0", (128, _NSUB), f32)
    dt_in("ms1", (128, _NSUB), f32)
    dt_in("w0", (64, 128), f32)
    for nm in ["w1p", "w2p", "w3ap", "w3bp"]:
        dt_in(nm, (128, 128), f32)
    for nm in ["b0c", "b1c", "b2c", "b3c"]:
        dt_in(nm, (128, 1), f32)
    dt_in("tw1", (64, 64), f32)
    dt_in("tw2", (64, 64), f32)
    dt_in("tb1", (64, 1), f32)
    dt_in("tb2", (64, 1), f32)
    dt_in("e0row", (1, 64), f32)
    dt_in("e1row", (1, 64), f32)
    dt_in("rsin", (32, 1), f32)
    dt_in("rcos", (32, 1), f32)
    dt_in("ones1", (1, 128), f32)
    out_d = nc.dram_tensor("out", [_SLP, 64], f32, kind="ExternalOutput")

    wnames = ["w1p", "w2p", None]
    bnames = ["b0c", "b1c", "b2c", "b3c"]
    ncw_max = int(meta.ncw.max())
    npw_max = max(len(meta.pairs_by_w[w]) for w in range(_NWIN))
    pair_gbase = [0]
    for w in range(_NWIN):
        pair_gbase.append(pair_gbase[-1] + len(meta.pairs_by_w[w]))

    def _interleave(dram_tile, row0, nst, F):
        ap = dram_tile[:]
        return bass.AP(ap.tensor, ap.offset + row0 * F,
                       [[F, 128], [128 * F, nst], [1, F]])

    qctr = [0]
    def next_q():
        q = qctr[0] % 4
        qctr[0] += 1
        return q

    # calls grouped per window for issue order
    calls_by_w = [[] for _ in range(_NWIN)]
    for c in meta.calls:
        calls_by_w[c[0]].append(c)

    with tile.TileContext(nc) as tc:
        with tc.tile_pool(name="consts", bufs=1) as cp, \
             tc.tile_pool(name="meta", bufs=1) as mp, \
             tc.tile_pool(name="dram", bufs=1, space="DRAM") as dram, \
             tc.tile_pool(name="g", bufs=3) as gp, \
             tc.tile_pool(name="oh", bufs=2) as ohp, \
             tc.tile_pool(name="small", bufs=4) as sp, \
             tc.tile_pool(name="xt", bufs=2) as xtp, \
             tc.tile_pool(name="psA", bufs=2, space="PSUM") as psA, \
             tc.tile_pool(name="psB", bufs=3, space="PSUM") as psB, \
             tc.tile_pool(name="psC", bufs=2, space="PSUM") as psC:

            def load(name, shape, dt, pool=cp):
                t = pool.tile(list(shape), dt, tag=name, name=name)
                nc.sync.dma_start(out=t[:], in_=din[name].ap())
                return t

            idx_t = load("midx", (128, meta.s_total // 16), mybir.dt.int16, mp)
            srhs_t = load("msrhs", (128, _NSUB * 128), f16, mp)
            self_t = load("mself", (128, _NSUB), f32)
            s0_t = load("ms0", (128, _NSUB), f32)
            s1_t = load("ms1", (128, _NSUB), f32)
            w0_t = load("w0", (64, 128), f32)
            wl_t = {nm: load(nm, (128, 128), f32) for nm in ["w1p", "w2p", "w3ap", "w3bp"]}
            b_t = {nm: load(nm, (128, 1), f32) for nm in bnames}
            tw1_t = load("tw1", (64, 64), f32)
            tw2_t = load("tw2", (64, 64), f32)
            tb1_t = load("tb1", (64, 1), f32)
            tb2_t = load("tb2", (64, 1), f32)
            e0_t = load("e0row", (1, 64), f32)
            e1_t = load("e1row", (1, 64), f32)
            rsin_t = load("rsin", (32, 1), f32)
            rcos_t = load("rcos", (32, 1), f32)
            ones1_t = load("ones1", (1, 128), f32)

            self_h = cp.tile([128, _NSUB], f16, tag="selfh", name="selfh")
            nc.vector.tensor_copy(self_h[:], self_t[:])
            eye_t = cp.tile([128, 128], f32, tag="eye", name="eye")
            make_identity(nc, eye_t[:])
            eye_h = cp.tile([128, 128], f16, tag="eyeh", name="eyeh")
            nc.vector.tensor_copy(eye_h[:], eye_t[:])

            # zero the gather buffers once (stale tails feed 0-masked matmuls)
            for _z in range(3):
                gz = gp.tile([128, ncw_max, _F], f16, tag="g", name="g")
                nc.vector.memset(gz[:], 0)

            # ---- DRAM working buffers ----
            slice_d = [[dram.tile([_CSIZE[c], _F], f16, tag=f"sl{l}_{c}",
                                  name=f"sl{l}_{c}") for c in range(4)]
                       for l in range(4)]
            full_d = [[dram.tile([_CSIZE[c] * _NCORES, _F], f16, tag=f"fu{l}_{c}",
                                 name=f"fu{l}_{c}") for c in range(4)]
                      for l in range(4)]
            h0T_d = dram.tile([128, _SLP], f32, tag="h0T", name="h0T")

            def ag(l, c):
                nc.gpsimd.collective_compute(
                    "AllGather", mybir.AluOpType.bypass,
                    replica_groups=[list(range(_NCORES))],
                    ins=[slice_d[l][c].opt()], outs=[full_d[l][c].opt()],
                )

            # window w rows [w*512, w*512+ws) -> quarter helpers
            def quarter_of_row(r):
                for c in range(4):
                    if r < _CSTART[c + 1]:
                        return c
                raise AssertionError

            def write_slice(l, row0, nst, src_ap):
                # rows [row0, row0+nst*128) always lie inside one quarter
                c = quarter_of_row(row0)
                assert row0 + nst * 128 <= _CSTART[c + 1]
                nc.sync.dma_start(
                    out=_interleave(slice_d[l][c], row0 - _CSTART[c], nst, _F),
                    in_=src_ap)

            def read_slice(l, row0, nst, dst_ap):
                c = quarter_of_row(row0)
                nc.sync.dma_start(
                    out=dst_ap,
                    in_=_interleave(slice_d[l][c], row0 - _CSTART[c], nst, _F))

            # ---- timestep embedding ----
            sc_t = sp.tile([64, 1], f32, tag="tsc", name="tsc")
            nc.scalar.activation(sc_t[:32, :], rsin_t[:], AT.Sin)
            nc.scalar.activation(sc_t[32:64, :], rcos_t[:], AT.Sin)
            h1ps = psC.tile([64, 1], f32, tag="temb", name="h1ps")
            nc.tensor.matmul(h1ps[:], lhsT=tw1_t[:], rhs=sc_t[:], start=True, stop=True)
            h1_t = sp.tile([64, 1], f32, tag="th1", name="th1")
            nc.scalar.activation(h1_t[:], h1ps[:], AT.Silu, bias=tb1_t[:, :1])
            t2ps = psC.tile([64, 1], f32, tag="temb", name="t2ps")
            nc.tensor.matmul(t2ps[:], lhsT=tw2_t[:], rhs=h1_t[:], start=True, stop=True)
            tembT = sp.tile([64, 1], f32, tag="tembT", name="tembT")
            nc.vector.tensor_scalar(out=tembT[:], in0=t2ps[:], scalar1=tb2_t[:, :1],
                                    scalar2=None, op0=OP.add)
            trow_ps = psC.tile([1, 64], f32, tag="temb", name="trow_ps")
            nc.tensor.transpose(trow_ps[:], in_=tembT[:], identity=eye_t[:64, :64])
            trow_t = sp.tile([1, 64], f32, tag="trowS", name="trowS")
            nc.vector.tensor_copy(trow_t[:], trow_ps[:])
            rows_ps = psC.tile([128, 192], f32, tag="temb", name="rows_ps")
            nc.tensor.matmul(rows_ps[:, 0:64], lhsT=ones1_t[:], rhs=trow_t[:],
                             start=True, stop=True, skip_group_check=True)
            nc.tensor.matmul(rows_ps[:, 64:128], lhsT=ones1_t[:], rhs=e0_t[:],
                             start=True, stop=True, skip_group_check=True)
            nc.tensor.matmul(rows_ps[:, 128:192], lhsT=ones1_t[:], rhs=e1_t[:],
                             start=True, stop=True, skip_group_check=True)
            addrows = cp.tile([128, 192], f32, tag="addrows", name="addrows")
            nc.vector.tensor_copy(addrows[:], rows_ps[:])

            # ---- x0 phase: Hs0 = dinv * ((noise + temb + lab) @ w0), batched x4 ----
            groups = [(g0 * 4, min(4, _NSUB - g0 * 4)) for g0 in range((_NSUB + 3) // 4)]
            ag0_done = 0
            for (st0, ng) in groups:
                nz = sp.tile([128, ng, 64], f32, tag="nz", name="nz")
                nap = din["noise"].ap()
                nc.sync.dma_start(
                    out=nz[:],
                    in_=bass.AP(nap.tensor, nap.offset + st0 * 128 * _D,
                                [[_D, 128], [128 * _D, ng], [1, _D]]))
                x0 = sp.tile([128, ng, 64], f32, tag="x0", name="x0")
                tr_b = bass.AP(addrows[:].tensor, addrows[:].offset,
                               [list(addrows[:].ap[0]), [0, ng], [1, 64]])
                nc.vector.tensor_tensor(out=x0[:], in0=nz[:], in1=tr_b, op=OP.add)
                lab = sp.tile([128, ng, 64], f32, tag="lab", name="lab")
                e0_b = bass.AP(addrows[:].tensor, addrows[:].offset + 64,
                               [list(addrows[:].ap[0]), [0, ng], [1, 64]])
                nc.vector.tensor_tensor(
                    out=lab[:], in0=e0_b,
                    in1=s0_t[:, st0:st0 + ng][:, :, None].to_broadcast([128, ng, 64]),
                    op=OP.mult)
                nc.vector.tensor_add(x0[:], x0[:], lab[:])
                e1_b = bass.AP(addrows[:].tensor, addrows[:].offset + 128,
                               [list(addrows[:].ap[0]), [0, ng], [1, 64]])
                nc.vector.tensor_tensor(
                    out=lab[:], in0=e1_b,
                    in1=s1_t[:, st0:st0 + ng][:, :, None].to_broadcast([128, ng, 64]),
                    op=OP.mult)
                nc.vector.tensor_add(x0[:], x0[:], lab[:])
                hs0 = sp.tile([128, ng, _F], f16, tag="hsout", name="hsout")
                for j in range(ng):
                    x0T_ps = psB.tile([64, 128], f32, tag="mm128", name="x0T_ps")
                    nc.tensor.transpose(x0T_ps[:], in_=x0[:, j, :], identity=eye_t[:])
                    x0T = sp.tile([64, 128], f32, tag="x0Ts", name="x0Ts")
                    nc.vector.tensor_copy(x0T[:], x0T_ps[:])
                    hps = psB.tile([128, 128], f32, tag="mm128", name="hps")
                    nc.tensor.matmul(hps[:], lhsT=x0T[:], rhs=w0_t[:], start=True, stop=True)
                    nc.vector.tensor_scalar(
                        out=hs0[:, j, :], in0=hps[:],
                        scalar1=self_t[:, st0 + j:st0 + j + 1], scalar2=None, op0=OP.mult)
                write_slice(0, st0 * 128, ng, hs0[:])
                # chunked AG0 as quarters complete (quarter ends at subtile 24/48/72/98)
                done_rows = (st0 + ng) * 128
                while ag0_done < 4 and done_rows >= _CSTART[ag0_done + 1]:
                    ag(0, ag0_done)
                    ag0_done += 1

            # ---- layers ----
            for layer in range(4):
                ag_next = 0
                for w in range(_NWIN):
                    ws = min(_WIN, _SLP - w * _WIN)
                    nst = ws // 128
                    ncw = int(meta.ncw[w])
                    npw = len(meta.pairs_by_w[w])
                    g = gp.tile([128, ncw_max, _F], f16, tag="g", name="g")
                    for (_, b, goff, csz, c0, nch) in calls_by_w[w]:
                        nc.gpsimd.dma_gather(
                            out_ap=g[:, c0:c0 + nch, :],
                            in_ap=full_d[layer][b][:],
                            idxs_ap=idx_t[:, goff // 16: goff // 16 + csz // 16],
                            num_idxs=csz, num_idxs_reg=csz, elem_size=_F,
                            queue_num=next_q(), single_packet=False,
                        )
                    # launch next layer's AG chunks once prior windows wrote them
                    if layer < 3 and ag_next < 4 and w * _WIN >= _CSTART[ag_next + 1] + 1024:
                        ag(layer + 1, ag_next)
                        ag_next += 1
                    # host-built one-hots, streamed from DRAM
                    oh = ohp.tile([128, npw_max, 128], f16, tag="oh", name="oh")
                    if npw:
                        nc.sync.dma_start(
                            out=oh[:, 0:npw, :],
                            in_=din["moh"].ap()[:, pair_gbase[w] * 128:
                                                (pair_gbase[w] + npw) * 128])
                    agg = psA.tile([128, ws], f32, tag="agg", name="agg")
                    for i, p in enumerate(meta.pairs_by_w[w]):
                        _, c_local, st_c, _, _, _ = p
                        nc.tensor.matmul(agg[:, st_c * 128:(st_c + 1) * 128],
                                         lhsT=g[:, c_local, :], rhs=oh[:, i, :],
                                         start=(i == 0), stop=False,
                                         skip_group_check=True)
                    # self-loop terms (diag rhs resident in SBUF)
                    hsb = sp.tile([128, nst, _F], f16, tag="hself", name="hself")
                    read_slice(layer, w * _WIN, nst, hsb[:])
                    for st in range(nst):
                        nc.tensor.matmul(agg[:, st * 128:(st + 1) * 128],
                                         lhsT=hsb[:, st, :],
                                         rhs=srhs_t[:, (4 * w + st) * 128:
                                                    (4 * w + st + 1) * 128],
                                         start=(npw == 0 and st == 0),
                                         stop=(st == nst - 1),
                                         skip_group_check=True)
                    xT = xtp.tile([128, ws], f32, tag="xT", name="xT")
                    nc.scalar.activation(xT[:], agg[:], AT.Silu, bias=b_t[bnames[layer]][:, :1])
                    if layer == 0:
                        nc.sync.dma_start(out=h0T_d[:, w * _WIN:w * _WIN + ws], in_=xT[:])
                    if layer < 3:
                        hps = psB.tile([128, ws], f32, tag="mm128", name="hps2")
                        if layer == 2:
                            h0b = sp.tile([128, ws], f32, tag="h0tile", name="h0tile")
                            nc.sync.dma_start(out=h0b[:],
                                              in_=h0T_d[:, w * _WIN:w * _WIN + ws])
                        for st in range(nst):
                            if layer < 2:
                                nc.tensor.matmul(hps[:, st * 128:(st + 1) * 128],
                                                 lhsT=xT[:, st * 128:(st + 1) * 128],
                                                 rhs=wl_t[wnames[layer]][:],
                                                 start=(st == 0), stop=False,
                                                 skip_group_check=True)
                            else:
                                nc.tensor.matmul(hps[:, st * 128:(st + 1) * 128],
                                                 lhsT=xT[:, st * 128:(st + 1) * 128],
                                                 rhs=wl_t["w3ap"][:], start=(st == 0), stop=False,
                                                 skip_group_check=True)
                                nc.tensor.matmul(hps[:, st * 128:(st + 1) * 128],
                                                 lhsT=h0b[:, st * 128:(st + 1) * 128],
                                                 rhs=wl_t["w3bp"][:],
                                                 start=False, stop=False, skip_group_check=True)
                        hsout = sp.tile([128, nst, _F], f16, tag="hsout", name="hsout")
                        nc.vector.tensor_tensor(
                            out=hsout[:], in0=hps[:].rearrange("p (s f) -> p s f", s=nst),
                            in1=self_h[:, 4 * w:4 * w + nst][:, :, None].to_broadcast([128, nst, _F]),
                            op=OP.mult)
                        write_slice(layer + 1, w * _WIN, nst, hsout[:])
                    else:
                        for st in range(ws // 128):
                            ops = psB.tile([128, 128], f32, tag="mm128", name="ops")
                            nc.tensor.transpose(ops[:], in_=xT[:, st * 128:(st + 1) * 128],
                                                identity=eye_t[:])
                            oc = sp.tile([128, 64], f32, tag="outt", name="outt")
                            nc.vector.tensor_copy(oc[:], ops[:, 0:64])
                            nc.sync.dma_start(
                                out=out_d.ap()[(w * _WIN + st * 128):(w * _WIN + st * 128 + 128), :],
                                in_=oc[:])
                # tail AG chunks for the next layer
                if layer < 3:
                    while ag_next < 4:
                        ag(layer + 1, ag_next)
                        ag_next += 1

    nc.compile()
    return nc


def _get_compiled(inputs):
    in_maps, meta, npairs_g = _prep(inputs)
    key = meta.alloc.tobytes()
    if key not in _compiled:
        _compiled[key] = _build(meta, npairs_g)
    return _compiled[key], in_maps


def _run(inputs, trace=False):
    _install_profile_shim()
    from concourse import bass_utils
    nc, in_maps = _get_compiled(inputs)
    res = bass_utils.run_bass_kernel_spmd(
        nc, in_maps, core_ids=list(range(_NCORES)), trace=trace)
    out = np.concatenate([res.results[k]["out"][:_SL] for k in range(_NCORES)], axis=0)
    return out[:_N].astype(np.float32), res.exec_time_ns


def kernel(**inputs):
    out, _ = _run(inputs, trace=False)
    return out


# revision 9
# speedup vs baseline: 1.3409x; 1.1042x over previous
"""Trainium2 Bass kernel for the 4-layer GCN diffusion denoiser (gnn_message_passing).

Strategy (8 NeuronCores, SPMD single program):
  - Nodes sharded 12500/core (padded to 12544 = 98*128). Edges routed to the core
    owning their dst node, grouped into 512-node windows.
  - Per layer, per-node features Hs = dinv * (X @ W) stored as fp16 [*, 128] rows,
    AllGather'ed across cores in FOUR quarter-slice chunks so the collective
    overlaps the previous layer's window compute. Gather bucket b = quarter b of
    every core's slice (view rows core*csize+local, int16-addressable).
  - Aggregation per window: descriptor-lean dma_gather streams (runs padded to
    x16 with dummy row-0 idxs, uniform across cores), segment-sum via PE matmuls
    against one-hot matrices built in a single fused DVE pass
    (iota==dstloc)*dinv_dst, self-loop via Hs_self x diag(dinv), Silu on ScalarE.
  - The next layer's H-matmul consumes the transposed activation tile directly.
"""

import math
import sys
import types

import numpy as np

_N, _E, _D, _G = 100000, 1000000, 64, 128
_NCORES = 8
_SL = _N // _NCORES          # 12500 real nodes per core
_SLP = 12544                 # padded per-core slice (98*128)
_NP = _SLP * _NCORES
_WIN = 512
_NWIN = (_SLP + _WIN - 1) // _WIN    # 25 windows (last is 256 nodes)
_NSUB = _SLP // 128          # 98 sub-tiles of 128 nodes
_F = 128
_PAD_DST = 1000.0
_CSTART = [0, 3072, 6144, 9216, 12544]          # quarter starts (local rows)
_CSIZE = [3072, 3072, 3072, 3328]
_NBUCK = 4
_MAX_CALL = 512

_compiled = {}


def _install_profile_shim():
    """Register the NTFF profile hook missing from this image's antenv."""
    try:
        import antenv
        from trn_agent_boot.trn_boot import _ntff_profile_via_ctypes
    except ImportError:
        return
    if "antenv.axon_hooks" in sys.modules:
        return
    mod = types.ModuleType("antenv.axon_hooks")
    hook = _ntff_profile_via_ctypes("/opt/axon/libaxon_pjrt.so")
    mod.get_axon_ntff_profile_hook = lambda: hook
    mod.set_axon_ntff_profile_hook = lambda h: None
    sys.modules["antenv.axon_hooks"] = mod
    antenv.axon_hooks = mod


def _ceil16(x):
    return (x + 15) // 16 * 16


class _Meta:
    """Shared (cross-core) gather stream structure."""
    pass


def _layout(runlen_max):
    """Build the shared call/chunk/pair layout from per-(w,b,st) max run lens."""
    m = _Meta()
    alloc = np.zeros((_NWIN, _NBUCK, 4), np.int64)
    for w in range(_NWIN):
        nst = min(_WIN, _SLP - w * _WIN) // 128
        for b in range(_NBUCK):
            for st in range(nst):
                r = int(runlen_max[w, b, st])
                alloc[w, b, st] = _ceil16(r) if r else 0
    m.alloc = alloc
    # run start offsets within the (w,b) stream
    m.run_off = np.zeros((_NWIN, _NBUCK, 4), np.int64)
    m.wb_size = np.zeros((_NWIN, _NBUCK), np.int64)
    for w in range(_NWIN):
        for b in range(_NBUCK):
            acc = 0
            for st in range(4):
                m.run_off[w, b, st] = acc
                acc += int(alloc[w, b, st])
            m.wb_size[w, b] = acc
    # global stream offset per (w,b)
    m.wb_base = np.zeros((_NWIN, _NBUCK), np.int64)
    acc = 0
    for w in range(_NWIN):
        for b in range(_NBUCK):
            m.wb_base[w, b] = acc
            acc += int(m.wb_size[w, b])
    m.s_total = acc
    # calls: split each (w,b) stream into pieces <= _MAX_CALL (x16 sizes)
    # each call owns cdiv(size,128) chunks in the window's chunk grid
    m.calls = []            # (w, b, stream_off, size, c0_window_local, nch)
    m.ncw = np.zeros(_NWIN, np.int64)
    m.pairs = []            # (w, c_local, st, slot_lo, slot_hi, stream_pos_lo)
    for w in range(_NWIN):
        cloc = 0
        for b in range(_NBUCK):
            size = int(m.wb_size[w, b])
            off = 0
            while off < size:
                csz = min(_MAX_CALL, size - off)
                nch = (csz + 127) // 128
                m.calls.append((w, b, int(m.wb_base[w, b] + off), csz, cloc, nch))
                # spans of this call's chunks against the run layout
                for k in range(nch):
                    lo = off + k * 128
                    hi = min(off + (k + 1) * 128, size)
                    for st in range(4):
                        ra, rs = int(m.alloc[w, b, st]), int(m.run_off[w, b, st])
                        if ra == 0:
                            continue
                        s_lo, s_hi = max(lo, rs), min(hi, rs + ra)
                        if s_lo < s_hi:
                            m.pairs.append((w, cloc + k, st, s_lo - lo, s_hi - lo,
                                            int(m.wb_base[w, b] + s_lo)))
                cloc += nch
                off += csz
        m.ncw[w] = cloc
    # pair index per window-local order
    m.pairs_by_w = [[] for _ in range(_NWIN)]
    for p in m.pairs:
        m.pairs_by_w[p[0]].append(p)
    m.npairs = len(m.pairs)
    return m


def _prep(inputs):
    """Host-side metadata build: edge bucketing, gather stream, one-hot scalars."""
    src = np.asarray(inputs["edge_index"][0], dtype=np.int64)
    dst = np.asarray(inputs["edge_index"][1], dtype=np.int64)
    deg = np.bincount(dst, minlength=_N).astype(np.float32) + 1.0
    dinv = (1.0 / np.sqrt(deg)).astype(np.float32)

    core = dst // _SL
    loc = dst % _SL
    w_of = loc // _WIN
    st_of = (loc % _WIN) // 128
    dstloc = (loc % 128).astype(np.float32)
    score = src // _SL
    sloc = src % _SL
    buck = np.digitize(sloc, _CSTART[1:4])
    csize_a = np.asarray(_CSIZE)
    cstart_a = np.asarray(_CSTART[:4])
    vrow = score * csize_a[buck] + (sloc - cstart_a[buck])
    dinv_d = dinv[dst]

    key = (((core * _NWIN + w_of) * _NBUCK + buck) * 4 + st_of)
    order = np.argsort(key, kind="stable")
    key_s = key[order]
    vrow_s = vrow[order].astype(np.int16)
    dstloc_s = dstloc[order]
    dinvd_s = dinv_d[order]

    nkeys = _NCORES * _NWIN * _NBUCK * 4
    counts = np.bincount(key_s, minlength=nkeys)
    runlen = counts.reshape(_NCORES, _NWIN, _NBUCK, 4)
    meta = _layout(runlen.max(axis=0))
    run_start = np.zeros(nkeys + 1, np.int64)
    np.cumsum(counts, out=run_start[1:])

    # stream position of every edge (per its own core's stream)
    rank = np.arange(len(key_s)) - run_start[key_s]
    wb_base_e = meta.wb_base[w_of[order], buck[order]]
    run_off_e = meta.run_off[w_of[order], buck[order], st_of[order]]
    pos = wb_base_e + run_off_e + rank          # stream slot per edge
    core_s = core[order]

    # pair lookup (w, c_local, st) -> window-local pair idx; and per-pair stream base
    pair_idx = {}
    pair_of_slot = {}
    for w in range(_NWIN):
        for i, p in enumerate(meta.pairs_by_w[w]):
            pair_idx[(w, p[1], p[2])] = i
    # global pair id (flat over windows, window-major)
    pair_gbase = np.zeros(_NWIN + 1, np.int64)
    for w in range(_NWIN):
        pair_gbase[w + 1] = pair_gbase[w] + len(meta.pairs_by_w[w])
    npairs_g = int(pair_gbase[_NWIN])

    # map every edge to (global pair, slot in chunk):
    # stream pos -> (w,b) known; slot_in_chunk and chunk derived from call split.
    # Rebuild per-(w,b) call boundaries for vectorized mapping.
    call_off_l = {}
    for (w, b, goff, csz, c0, nch) in meta.calls:
        call_off_l.setdefault((w, b), []).append((goff, csz, c0))
    # per-edge: find its call via offset within (w,b)
    gpair = np.zeros(len(pos), np.int64)
    slotc = np.zeros(len(pos), np.int64)
    wb_rel = pos - wb_base_e
    w_e, b_e, st_e = w_of[order], buck[order], st_of[order]
    for (w, b), calls in call_off_l.items():
        sel = (w_e == w) & (b_e == b)
        if not sel.any():
            continue
        rel = wb_rel[sel]
        st_sel = st_e[sel]
        gp = np.zeros(len(rel), np.int64)
        sc = np.zeros(len(rel), np.int64)
        wbb = int(meta.wb_base[w, b])
        for (goff, csz, c0) in calls:
            off = goff - wbb
            m2 = (rel >= off) & (rel < off + csz)
            if not m2.any():
                continue
            r2 = rel[m2] - off
            ch = r2 // 128
            sl = r2 % 128
            # pair id from (w, c0+ch, st)
            pid = np.array([pair_idx[(w, int(c0 + c), int(s))]
                            for c, s in zip(ch, st_sel[m2])], np.int64)
            gp[m2] = pair_gbase[w] + pid
            sc[m2] = sl
        gpair[sel] = gp
        slotc[sel] = sc

    is_norm = np.zeros(_N, bool)
    is_norm[np.asarray(inputs["train_norm"])] = True
    is_anm = np.zeros(_N, bool)
    is_anm[np.asarray(inputs["train_anm"])] = True

    noise = np.asarray(inputs["noise_x"], np.float32)
    t_val = float(np.asarray(inputs["t"]).reshape(-1)[0])
    half = _D // 2
    freqs = np.exp(
        np.arange(half, dtype=np.float32) * (-math.log(10000.0) / (half - 1))
    ).astype(np.float32)

    w_ = {m: np.asarray(inputs[m], np.float32) for m in
          ["w0", "b0", "w1", "b1", "w2", "b2", "w3", "b3",
           "time_w1", "time_b1", "time_w2", "time_b2", "label_emb"]}
    w1p = np.zeros((128, 128), np.float32); w1p[:, :64] = w_["w1"]
    w2p = np.zeros((128, 128), np.float32); w2p[:64, :] = w_["w2"]
    w3ap = np.zeros((128, 128), np.float32); w3ap[:, :64] = w_["w3"][:128]
    w3bp = np.zeros((128, 128), np.float32); w3bp[:, :64] = w_["w3"][128:]
    b1p = np.zeros((128, 1), np.float32); b1p[:64, 0] = w_["b1"]
    b3p = np.zeros((128, 1), np.float32); b3p[:64, 0] = w_["b3"]

    shared = {
        "w0": w_["w0"],
        "w1p": w1p, "w2p": w2p, "w3ap": w3ap, "w3bp": w3bp,
        "b0c": w_["b0"].reshape(128, 1).astype(np.float32),
        "b1c": b1p,
        "b2c": w_["b2"].reshape(128, 1).astype(np.float32),
        "b3c": b3p,
        "tw1": w_["time_w1"], "tw2": w_["time_w2"],
        "tb1": w_["time_b1"].reshape(64, 1).astype(np.float32),
        "tb2": w_["time_b2"].reshape(64, 1).astype(np.float32),
        "e0row": w_["label_emb"][0].reshape(1, 64).astype(np.float32),
        "e1row": w_["label_emb"][1].reshape(1, 64).astype(np.float32),
        "rsin": (np.mod(t_val * freqs + np.pi, 2 * np.pi) - np.pi).reshape(32, 1).astype(np.float32),
        "rcos": (np.mod(t_val * freqs + np.pi / 2 + np.pi, 2 * np.pi) - np.pi).reshape(32, 1).astype(np.float32),
        "ones1": np.ones((1, 128), np.float32),
    }

    in_maps = []
    for k in range(_NCORES):
        mine = core_s == k
        idx_stream = np.zeros(meta.s_total, np.int16)   # dummies -> view row 0
        idx_stream[pos[mine]] = vrow_s[mine]
        import ml_dtypes
        f8 = ml_dtypes.float8_e4m3fn
        ohs = np.zeros((128, npairs_g, 128), f8)
        ohs[slotc[mine], gpair[mine], dstloc_s[mine].astype(np.int64)] = f8(1.0)

        wrapped = np.tile(idx_stream.reshape(-1, 16).T, (8, 1))

        nodes = np.arange(_SLP) + k * _SL
        nodes_c = np.minimum(nodes, _N - 1)
        sd = dinv[nodes_c].copy()
        sd[np.arange(_SLP) >= _SL] = 1.0
        selfdinv = sd.reshape(_NSUB, 128).T.copy()

        s_n = np.zeros(_SLP, np.float32)
        s_a = np.zeros(_SLP, np.float32)
        real = np.arange(_SLP) < _SL
        s_n[real] = is_norm[nodes[real]]
        s_a[real] = is_anm[nodes[real]] & ~is_norm[nodes[real]]

        nz = np.zeros((_SLP, _D), np.float32)
        nz[:_SL] = noise[k * _SL:(k + 1) * _SL]

        # dinv of each local node broadcast across all 128 partitions
        dibc = np.repeat(sd.astype(np.float16)[None, :], 128, axis=0)

        m = dict(shared)
        m.update({
            "midx": wrapped,
            "moh": ohs.reshape(128, npairs_g * 128),
            "mdibc": dibc,
            "mself": selfdinv,
            "ms0": s_n.reshape(_NSUB, 128).T.copy(),
            "ms1": s_a.reshape(_NSUB, 128).T.copy(),
            "noise": nz,
        })
        in_maps.append(m)

    return in_maps, meta, npairs_g


def _build(meta, npairs_g):
    import concourse.bass as bass
    import concourse.bacc as bacc
    import concourse.tile as tile
    from concourse import mybir
    from concourse.masks import make_identity

    f32 = mybir.dt.float32
    f16 = mybir.dt.float16
    AT = mybir.ActivationFunctionType
    OP = mybir.AluOpType

    nc = bacc.Bacc("TRN2", target_bir_lowering=False, debug=False,
                   num_devices=_NCORES, dynamic_dma_scratch_size=32768,
                   num_swdge_queues=4)

    din = {}
    def dt_in(name, shape, dt):
        din[name] = nc.dram_tensor(name, list(shape), dt, kind="ExternalInput")
        return din[name]

    dt_in("noise", (_SLP, _D), f32)
    f8 = mybir.dt.float8e4
    dt_in("midx", (128, meta.s_total // 16), mybir.dt.int16)
    dt_in("moh", (128, npairs_g * 128), f8)
    dt_in("mdibc", (128, _SLP), f16)
    dt_in("mself", (128, _NSUB), f32)
    dt_in("ms0", (128, _NSUB), f32)
    dt_in("ms1", (128, _NSUB), f32)
    dt_in("w0", (64, 128), f32)
    for nm in ["w1p", "w2p", "w3ap", "w3bp"]:
        dt_in(nm, (128, 128), f32)
    for nm in ["b0c", "b1c", "b2c", "b3c"]:
        dt_in(nm, (128, 1), f32)
    dt_in("tw1", (64, 64), f32)
    dt_in("tw2", (64, 64), f32)
    dt_in("tb1", (64, 1), f32)
    dt_in("tb2", (64, 1), f32)
    dt_in("e0row", (1, 64), f32)
    dt_in("e1row", (1, 64), f32)
    dt_in("rsin", (32, 1), f32)
    dt_in("rcos", (32, 1), f32)
    dt_in("ones1", (1, 128), f32)
    out_d = nc.dram_tensor("out", [_SLP, 64], f32, kind="ExternalOutput")

    wnames = ["w1p", "w2p", None]
    bnames = ["b0c", "b1c", "b2c", "b3c"]
    ncw_max = int(meta.ncw.max())
    npw_max = max(len(meta.pairs_by_w[w]) for w in range(_NWIN))
    pair_gbase = [0]
    for w in range(_NWIN):
        pair_gbase.append(pair_gbase[-1] + len(meta.pairs_by_w[w]))

    def _interleave(dram_tile, row0, nst, F):
        ap = dram_tile[:]
        return bass.AP(ap.tensor, ap.offset + row0 * F,
                       [[F, 128], [128 * F, nst], [1, F]])

    qctr = [0]
    def next_q():
        q = qctr[0] % 4
        qctr[0] += 1
        return q

    # calls grouped per window for issue order
    calls_by_w = [[] for _ in range(_NWIN)]
    for c in meta.calls:
        calls_by_w[c[0]].append(c)

    with tile.TileContext(nc) as tc:
        with tc.tile_pool(name="consts", bufs=1) as cp, \
             tc.tile_pool(name="meta", bufs=1) as mp, \
             tc.tile_pool(name="dram", bufs=1, space="DRAM") as dram, \
             tc.tile_pool(name="g", bufs=3) as gp, \
             tc.tile_pool(name="oh", bufs=2) as ohp, \
             tc.tile_pool(name="small", bufs=4) as sp, \
             tc.tile_pool(name="xt", bufs=2) as xtp, \
             tc.tile_pool(name="psA", bufs=2, space="PSUM") as psA, \
             tc.tile_pool(name="psB", bufs=3, space="PSUM") as psB, \
             tc.tile_pool(name="psC", bufs=2, space="PSUM") as psC:

            def load(name, shape, dt, pool=cp):
                t = pool.tile(list(shape), dt, tag=name, name=name)
                nc.sync.dma_start(out=t[:], in_=din[name].ap())
                return t

            idx_t = load("midx", (128, meta.s_total // 16), mybir.dt.int16, mp)
            dibc_t = load("mdibc", (128, _SLP), f16, mp)
            self_t = load("mself", (128, _NSUB), f32)
            s0_t = load("ms0", (128, _NSUB), f32)
            s1_t = load("ms1", (128, _NSUB), f32)
            w0_t = load("w0", (64, 128), f32)
            wl_t = {nm: load(nm, (128, 128), f32) for nm in ["w1p", "w2p", "w3ap", "w3bp"]}
            b_t = {nm: load(nm, (128, 1), f32) for nm in bnames}
            tw1_t = load("tw1", (64, 64), f32)
            tw2_t = load("tw2", (64, 64), f32)
            tb1_t = load("tb1", (64, 1), f32)
            tb2_t = load("tb2", (64, 1), f32)
            e0_t = load("e0row", (1, 64), f32)
            e1_t = load("e1row", (1, 64), f32)
            rsin_t = load("rsin", (32, 1), f32)
            rcos_t = load("rcos", (32, 1), f32)
            ones1_t = load("ones1", (1, 128), f32)

            self_h = cp.tile([128, _NSUB], f16, tag="selfh", name="selfh")
            nc.vector.tensor_copy(self_h[:], self_t[:])
            eye_t = cp.tile([128, 128], f32, tag="eye", name="eye")
            make_identity(nc, eye_t[:])
            eye_f8 = cp.tile([128, 128], f8, tag="eyef8", name="eyef8")
            nc.vector.tensor_copy(eye_f8[:], eye_t[:])

            # zero the gather buffers once (stale tails feed 0-masked matmuls)
            for _z in range(3):
                gz = gp.tile([128, ncw_max, _F], f16, tag="g", name="g")
                nc.vector.memset(gz[:], 0)

            # ---- DRAM working buffers ----
            slice_d = [[dram.tile([_CSIZE[c], _F], f16, tag=f"sl{l}_{c}",
                                  name=f"sl{l}_{c}") for c in range(4)]
                       for l in range(4)]
            full_d = [[dram.tile([_CSIZE[c] * _NCORES, _F], f16, tag=f"fu{l}_{c}",
                                 name=f"fu{l}_{c}") for c in range(4)]
                      for l in range(4)]
            h0T_d = dram.tile([128, _SLP], f32, tag="h0T", name="h0T")

            def ag(l, c):
                nc.gpsimd.collective_compute(
                    "AllGather", mybir.AluOpType.bypass,
                    replica_groups=[list(range(_NCORES))],
                    ins=[slice_d[l][c].opt()], outs=[full_d[l][c].opt()],
                )

            # window w rows [w*512, w*512+ws) -> quarter helpers
            def quarter_of_row(r):
                for c in range(4):
                    if r < _CSTART[c + 1]:
                        return c
                raise AssertionError

            def write_slice(l, row0, nst, src_ap):
                # rows [row0, row0+nst*128) always lie inside one quarter
                c = quarter_of_row(row0)
                assert row0 + nst * 128 <= _CSTART[c + 1]
                nc.sync.dma_start(
                    out=_interleave(slice_d[l][c], row0 - _CSTART[c], nst, _F),
                    in_=src_ap)

            def read_slice(l, row0, nst, dst_ap):
                c = quarter_of_row(row0)
                nc.sync.dma_start(
                    out=dst_ap,
                    in_=_interleave(slice_d[l][c], row0 - _CSTART[c], nst, _F))

            # ---- timestep embedding ----
            sc_t = sp.tile([64, 1], f32, tag="tsc", name="tsc")
            nc.scalar.activation(sc_t[:32, :], rsin_t[:], AT.Sin)
            nc.scalar.activation(sc_t[32:64, :], rcos_t[:], AT.Sin)
            h1ps = psC.tile([64, 1], f32, tag="temb", name="h1ps")
            nc.tensor.matmul(h1ps[:], lhsT=tw1_t[:], rhs=sc_t[:], start=True, stop=True)
            h1_t = sp.tile([64, 1], f32, tag="th1", name="th1")
            nc.scalar.activation(h1_t[:], h1ps[:], AT.Silu, bias=tb1_t[:, :1])
            t2ps = psC.tile([64, 1], f32, tag="temb", name="t2ps")
            nc.tensor.matmul(t2ps[:], lhsT=tw2_t[:], rhs=h1_t[:], start=True, stop=True)
            tembT = sp.tile([64, 1], f32, tag="tembT", name="tembT")
            nc.vector.tensor_scalar(out=tembT[:], in0=t2ps[:], scalar1=tb2_t[:, :1],
                                    scalar2=None, op0=OP.add)
            trow_ps = psC.tile([1, 64], f32, tag="temb", name="trow_ps")
            nc.tensor.transpose(trow_ps[:], in_=tembT[:], identity=eye_t[:64, :64])
            trow_t = sp.tile([1, 64], f32, tag="trowS", name="trowS")
            nc.vector.tensor_copy(trow_t[:], trow_ps[:])
            rows_ps = psC.tile([128, 192], f32, tag="temb", name="rows_ps")
            nc.tensor.matmul(rows_ps[:, 0:64], lhsT=ones1_t[:], rhs=trow_t[:],
                             start=True, stop=True, skip_group_check=True)
            nc.tensor.matmul(rows_ps[:, 64:128], lhsT=ones1_t[:], rhs=e0_t[:],
                             start=True, stop=True, skip_group_check=True)
            nc.tensor.matmul(rows_ps[:, 128:192], lhsT=ones1_t[:], rhs=e1_t[:],
                             start=True, stop=True, skip_group_check=True)
            addrows = cp.tile([128, 192], f32, tag="addrows", name="addrows")
            nc.vector.tensor_copy(addrows[:], rows_ps[:])

            # ---- x0 phase: Hs0 = dinv * ((noise + temb + lab) @ w0), batched x4 ----
            groups = [(g0 * 4, min(4, _NSUB - g0 * 4)) for g0 in range((_NSUB + 3) // 4)]
            ag0_done = 0
            for (st0, ng) in groups:
                nz = sp.tile([128, ng, 64], f32, tag="nz", name="nz")
                nap = din["noise"].ap()
                nc.sync.dma_start(
                    out=nz[:],
                    in_=bass.AP(nap.tensor, nap.offset + st0 * 128 * _D,
                                [[_D, 128], [128 * _D, ng], [1, _D]]))
                x0 = sp.tile([128, ng, 64], f32, tag="x0", name="x0")
                tr_b = bass.AP(addrows[:].tensor, addrows[:].offset,
                               [list(addrows[:].ap[0]), [0, ng], [1, 64]])
                nc.vector.tensor_tensor(out=x0[:], in0=nz[:], in1=tr_b, op=OP.add)
                lab = sp.tile([128, ng, 64], f32, tag="lab", name="lab")
                e0_b = bass.AP(addrows[:].tensor, addrows[:].offset + 64,
                               [list(addrows[:].ap[0]), [0, ng], [1, 64]])
                nc.vector.tensor_tensor(
                    out=lab[:], in0=e0_b,
                    in1=s0_t[:, st0:st0 + ng][:, :, None].to_broadcast([128, ng, 64]),
                    op=OP.mult)
                nc.vector.tensor_add(x0[:], x0[:], lab[:])
                e1_b = bass.AP(addrows[:].tensor, addrows[:].offset + 128,
                               [list(addrows[:].ap[0]), [0, ng], [1, 64]])
                nc.vector.tensor_tensor(
                    out=lab[:], in0=e1_b,
                    in1=s1_t[:, st0:st0 + ng][:, :, None].to_broadcast([128, ng, 64]),
                    op=OP.mult)
                nc.vector.tensor_add(x0[:], x0[:], lab[:])
                hs0 = sp.tile([128, ng, _F], f16, tag="hsout", name="hsout")
                for j in range(ng):
                    x0T_ps = psB.tile([64, 128], f32, tag="mm128", name="x0T_ps")
                    nc.tensor.transpose(x0T_ps[:], in_=x0[:, j, :], identity=eye_t[:])
                    x0T = sp.tile([64, 128], f32, tag="x0Ts", name="x0Ts")
                    nc.vector.tensor_copy(x0T[:], x0T_ps[:])
                    hps = psB.tile([128, 128], f32, tag="mm128", name="hps")
                    nc.tensor.matmul(hps[:], lhsT=x0T[:], rhs=w0_t[:], start=True, stop=True)
                    nc.vector.tensor_scalar(
                        out=hs0[:, j, :], in0=hps[:],
                        scalar1=self_t[:, st0 + j:st0 + j + 1], scalar2=None, op0=OP.mult)
                write_slice(0, st0 * 128, ng, hs0[:])
                # chunked AG0 as quarters complete (quarter ends at subtile 24/48/72/98)
                done_rows = (st0 + ng) * 128
                while ag0_done < 4 and done_rows >= _CSTART[ag0_done + 1]:
                    ag(0, ag0_done)
                    ag0_done += 1

            # ---- layers ----
            for layer in range(4):
                ag_next = 0
                for w in range(_NWIN):
                    ws = min(_WIN, _SLP - w * _WIN)
                    nst = ws // 128
                    ncw = int(meta.ncw[w])
                    npw = len(meta.pairs_by_w[w])
                    g = gp.tile([128, ncw_max, _F], f16, tag="g", name="g")
                    for (_, b, goff, csz, c0, nch) in calls_by_w[w]:
                        nc.gpsimd.dma_gather(
                            out_ap=g[:, c0:c0 + nch, :],
                            in_ap=full_d[layer][b][:],
                            idxs_ap=idx_t[:, goff // 16: goff // 16 + csz // 16],
                            num_idxs=csz, num_idxs_reg=csz, elem_size=_F,
                            queue_num=next_q(), single_packet=False,
                        )
                    # launch next layer's AG chunks once prior windows wrote them
                    if layer < 3 and ag_next < 4 and w * _WIN >= _CSTART[ag_next + 1] + 1024:
                        ag(layer + 1, ag_next)
                        ag_next += 1
                    # host-built one-hots, streamed from DRAM
                    oh = ohp.tile([128, npw_max, 128], f8, tag="oh", name="oh")
                    if npw:
                        nc.sync.dma_start(
                            out=oh[:, 0:npw, :],
                            in_=din["moh"].ap()[:, pair_gbase[w] * 128:
                                                (pair_gbase[w] + npw) * 128])
                    agg = psA.tile([128, ws], f32, tag="agg", name="agg")
                    for i, p in enumerate(meta.pairs_by_w[w]):
                        _, c_local, st_c, _, _, _ = p
                        nc.tensor.matmul(agg[:, st_c * 128:(st_c + 1) * 128],
                                         lhsT=g[:, c_local, :], rhs=oh[:, i, :],
                                         start=(i == 0), stop=False,
                                         skip_group_check=True)
                    # self-loop terms (identity rhs; dinv_dst applied post-agg)
                    hsb = sp.tile([128, nst, _F], f16, tag="hself", name="hself")
                    read_slice(layer, w * _WIN, nst, hsb[:])
                    for st in range(nst):
                        nc.tensor.matmul(agg[:, st * 128:(st + 1) * 128],
                                         lhsT=hsb[:, st, :],
                                         rhs=eye_f8[:],
                                         start=(npw == 0 and st == 0),
                                         stop=(st == nst - 1),
                                         skip_group_check=True)
                    aggs = sp.tile([128, ws], f32, tag="aggs", name="aggs")
                    nc.vector.tensor_tensor(
                        out=aggs[:], in0=agg[:],
                        in1=dibc_t[:, w * _WIN:w * _WIN + ws], op=OP.mult)
                    xT = xtp.tile([128, ws], f32, tag="xT", name="xT")
                    nc.scalar.activation(xT[:], aggs[:], AT.Silu, bias=b_t[bnames[layer]][:, :1])
                    if layer == 0:
                        nc.sync.dma_start(out=h0T_d[:, w * _WIN:w * _WIN + ws], in_=xT[:])
                    if layer < 3:
                        hps = psB.tile([128, ws], f32, tag="mm128", name="hps2")
                        if layer == 2:
                            h0b = sp.tile([128, ws], f32, tag="h0tile", name="h0tile")
                            nc.sync.dma_start(out=h0b[:],
                                              in_=h0T_d[:, w * _WIN:w * _WIN + ws])
                        for st in range(nst):
                            if layer < 2:
                                nc.tensor.matmul(hps[:, st * 128:(st + 1) * 128],
                                                 lhsT=xT[:, st * 128:(st + 1) * 128],
                                                 rhs=wl_t[wnames[layer]][:],
                                                 start=(st == 0), stop=False,
                                                 skip_group_check=True)
                            else:
                                nc.tensor.matmul(hps[:, st * 128:(st + 1) * 128],
                                                 lhsT=xT[:, st * 128:(st + 1) * 128],
                                                 rhs=wl_t["w3ap"][:], start=(st == 0), stop=False,
                                                 skip_group_check=True)
                                nc.tensor.matmul(hps[:, st * 128:(st + 1) * 128],
                                                 lhsT=h0b[:, st * 128:(st + 1) * 128],
                                                 rhs=wl_t["w3bp"][:],
                                                 start=False, stop=False, skip_group_check=True)
                        hsout = sp.tile([128, nst, _F], f16, tag="hsout", name="hsout")
                        nc.vector.tensor_tensor(
                            out=hsout[:], in0=hps[:].rearrange("p (s f) -> p s f", s=nst),
                            in1=self_h[:, 4 * w:4 * w + nst][:, :, None].to_broadcast([128, nst, _F]),
                            op=OP.mult)
                        write_slice(layer + 1, w * _WIN, nst, hsout[:])
                    else:
                        for st in range(ws // 128):
                            ops = psB.tile([128, 128], f32, tag="mm128", name="ops")
                            nc.tensor.transpose(ops[:], in_=xT[:, st * 128:(st + 1) * 128],
                                                identity=eye_t[:])
                            oc = sp.tile([128, 64], f32, tag="outt", name="outt")
                            nc.vector.tensor_copy(oc[:], ops[:, 0:64])
                            nc.sync.dma_start(
                                out=out_d.ap()[(w * _WIN + st * 128):(w * _WIN + st * 128 + 128), :],
                                in_=oc[:])
                # tail AG chunks for the next layer
                if layer < 3:
                    while ag_next < 4:
                        ag(layer + 1, ag_next)
                        ag_next += 1

    nc.compile()
    return nc


def _get_compiled(inputs):
    in_maps, meta, npairs_g = _prep(inputs)
    key = meta.alloc.tobytes()
    if key not in _compiled:
        _compiled[key] = _build(meta, npairs_g)
    return _compiled[key], in_maps


def _run(inputs, trace=False):
    _install_profile_shim()
    from concourse import bass_utils
    nc, in_maps = _get_compiled(inputs)
    res = bass_utils.run_bass_kernel_spmd(
        nc, in_maps, core_ids=list(range(_NCORES)), trace=trace)
    out = np.concatenate([res.results[k]["out"][:_SL] for k in range(_NCORES)], axis=0)
    return out[:_N].astype(np.float32), res.exec_time_ns


def kernel(**inputs):
    out, _ = _run(inputs, trace=False)
    return out
